# revision 48
# baseline (speedup 1.0000x reference)
"""Trainium2 Bass kernel for nn_Condensation: 10 sequential masked-Gaussian-blur
composites over a [16,3,768,768] image, data-parallel over 8 NeuronCores.

v22 strategy (per core, 2 images = 6 image-channels):
  - Row-offset block grid (delta chosen so EVERY drop's mask support fits in
    exactly 2 h-blocks of 128 rows): ~25% less elementwise/matmul work than a
    0-based grid and no false inter-drop deps from block padding.
  - Drops emitted in exact topological order of the true spatial-overlap DAG
    (non-overlapping drops commute): 3 levels of 4/4/2 drops.
  - Single continuous software pipeline over all 30 (drop, channel-pair)
    units at skew 3 (comp[u-3] <- evict/q/passB[u-1] <- om/passA[u]), using
    an emission order that keeps every drop dependence >= 2 positions apart
    (falls back to skew 2 if impossible); PSUM rings (2 psa + 2 psb bufs =
    8 banks) recycle only after their reader is emitted.
  - State in SBUF as bf16 [128, 6, NB, 768]; only rows any drop touches are
    loaded/stored (partial-partition edge blocks, zero-padded fat loads: one
    DMA per (pair, block-half)).
  - Masks materialized per (jj, h-block) as plain strided slices (stride-0
    broadcast APs lose the DVE fast path); no +-p w-margins anywhere
    (sources outside [w0,w1) are mask-zero). All loads ride ONE queue in
    first-use order: params of first 4 drops, image halves, rest.
  - Separable blur: two banded bf16 matmul passes on TensorE (f32 PSUM);
    pass B streams the full union band on the first w-chunk (start=True
    zeroes psb), the true kh support on the second, then a third matmul
    accumulates -I @ state so psb = blur - out ARRIVES pre-subtracted.
  - Composite is therefore just t2 = m*psb and an in-place out += t2 (the
    old q = out - om precompute and its slow strided reads are gone --
    ~25% of the Vector/GpSimd elementwise work moved onto idle TensorE
    capacity, exactly, in f32 PSUM). Per-op greedy balance across
    Vector/Scalar/GpSimd with HW-trace-calibrated contended costs.
  - Stores split per (block, w-piece, channel-pair), each issued as soon as
    its piece's last-writer drop composites that pair, so output DMA drains
    throughout instead of at the end.
"""
import numpy as np
import ml_dtypes

NUM_DROPS = 10
MIN_R, MAX_R = 60.0, 80.0
BETA = 1.8
BLUR_RADII = [11.3535, 17.9381, 5.7966, 10.8586, 5.5301, 15.9075, 12.3225, 13.4871, 6.6639, 9.5413]


def _ksize(r):
    k = int(2 * r) + 1
    return k + 1 if k % 2 == 0 else k


KSIZES = [_ksize(r) for r in BLUR_RADII]
H = W = 768
B_TOTAL, C = 16, 3
N_CORES = 8
B_LOC = B_TOTAL // N_CORES          # 2 images per core
IC = B_LOC * C                      # 6 image-channels per core
NG = IC // 2                        # 3 pairs of image-channels
P = 128
EPS = 5e-3                          # mask support threshold (error-validated)

_bf16 = ml_dtypes.bfloat16
_fp8 = ml_dtypes.float8_e4m3fn


def _conv_matrix(sigma, ksize, n=768):
    """n x n matrix Kmat with blur_1d(x) = Kmat @ x, matching the reference
    (correlation with normalized gaussian, 'reflect' padding)."""
    half = (ksize - 1) * 0.5
    xs = np.linspace(-half, half, ksize)
    pdf = np.exp(-0.5 * (xs / np.float64(sigma)) ** 2)
    k1 = (pdf / pdf.sum()).astype(np.float32).astype(np.float64)
    pad = ksize // 2
    Kmat = np.zeros((n, n), dtype=np.float64)
    idx = np.arange(n)[:, None] + np.arange(ksize)[None, :] - pad
    idx = np.abs(idx)
    idx = np.where(idx >= n, 2 * n - 2 - idx, idx)
    np.add.at(Kmat, (np.repeat(np.arange(n), ksize), idx.ravel()),
              np.tile(k1, n))
    return Kmat.astype(np.float32)


class _Drop:
    pass


def _drop_meta(positions, radius):
    """Host-side per-drop geometry + tensors (shared across cores) on the
    row-offset block grid."""
    pos = np.clip(np.asarray(positions, np.float32), -1.0, 1.0)
    rad = np.clip(np.asarray(radius, np.float32), MIN_R, MAX_R)
    s = float(np.sqrt((-np.log(EPS)) ** (1.0 / BETA)))
    s2 = s * s

    geo = []
    for j in range(NUM_DROPS):
        x0 = (pos[j, 0] + 1.0) / 2.0 * W
        y0 = (pos[j, 1] + 1.0) / 2.0 * H
        wr = rad[j]
        hr = wr * np.float32(0.8)
        p = KSIZES[j] // 2
        h0 = max(0, int(np.floor(y0 - s * hr))) & ~1
        h1 = min(H, (int(np.ceil(y0 + s * hr)) + 2) & ~1)
        w0 = max(0, int(np.floor(x0 - s * wr))) & ~1
        w1 = min(W, (int(np.ceil(x0 + s * wr)) + 2) & ~1)
        geo.append([h0, h1, w0, w1, p, float(x0), float(y0), float(wr), float(hr)])

    # pick an even grid offset so every drop spans exactly 2 blocks
    delta = None
    for dd_ in range(0, 128, 2):
        if all(((g[0] - dd_) % 128) + (g[1] - g[0]) <= 256 for g in geo):
            delta = dd_
            break
    assert delta is not None, "no 2-block grid offset exists"
    hmin = min(g[0] for g in geo)
    hmax = max(g[1] for g in geo)
    g0 = hmin - ((hmin - delta) % 128)
    NB = -((g0 - hmax) // 128)

    drops = []
    for j in range(NUM_DROPS):
        h0, h1, w0, w1, p, x0, y0, wr, hr = geo[j]
        d = _Drop()
        d.j, d.p = j, p
        d.B0 = (h0 - g0) // 128
        d.HBs = g0 + 128 * d.B0
        assert h1 - d.HBs <= 256 and d.B0 + 2 <= NB
        # cap w so Wt <= 256 (two overlapping 128-col chunks)
        wcap = 256 - 2 * p - 2
        while w1 - w0 > wcap:
            if x0 - w0 > w1 - x0:
                w0 += 2
            else:
                w1 -= 2
        d.h0, d.h1, d.w0, d.w1 = h0, h1, w0, w1
        d.span = h1 - h0
        d.Wr = w1 - w0
        d.voff = h0 - d.HBs
        # no margin: sources outside [w0,w1) are mask-zero, so the
        # horizontal pass can contract over [w0,w1) only
        wa, wb = w0, w1
        d.wa, d.wb = wa, wb
        d.Wt = wb - wa
        assert d.Wt <= 256 and d.span <= 256
        d.WBn = (d.Wt + P - 1) // P
        assert d.WBn == 2
        d.cstarts = [wa, wb - P]

        # pass A bands per k-block: output h' range (relative to h0)
        d.bandsA = []
        for k in range(2):
            a = max(0, d.HBs + P * k - p - h0)
            b = min(d.span, d.HBs + P * (k + 1) + p - h0)
            d.bandsA.append((a, b))

        # per h-block composite w-range [wl, wr) from the ellipse extent
        d.hbw = []
        for hb in range(2):
            ra = max(h0, d.HBs + P * hb)
            rb = min(h1, d.HBs + P * (hb + 1))
            if ra - 1 < y0 < rb:
                dh = 0.0
            else:
                dh = min(abs(ra - y0), abs(rb - 1 - y0))
            half = wr * np.sqrt(max(0.0, s2 - (dh / hr) ** 2))
            wl = max(w0, (int(np.floor(x0 - half)) - 2) & ~1)
            wr_ = min(w1, (int(np.ceil(x0 + half)) + 4) & ~1)
            wr_ = max(wr_, wl + 2)
            d.hbw.append((wl, wr_))
        # union composite window across both h-blocks (mask is zero outside
        # each block's own [wl, wr), so fused ops over the union are exact)
        d.wlu = min(wl for wl, _ in d.hbw)
        d.wru = max(wr_ for _, wr_ in d.hbw)

        # mask over [2 blocks of 128 rows] x [wa:wb], zero outside support
        rows = (d.HBs + np.arange(2 * P, dtype=np.int64)).astype(np.float32)
        dd = (rows[:, None] - y0) ** 2 / hr ** 2 + \
             (np.arange(wa, wb, dtype=np.float32)[None, :] - x0) ** 2 / wr ** 2
        m = np.clip(np.exp(-(dd.astype(np.float32) ** np.float32(BETA)) + np.float32(1e-10)), 0.0, 1.0)
        m = np.where(dd <= np.float32(s2), m, 0.0).astype(np.float32)
        mz = np.zeros_like(m)
        for hb in range(2):
            ra = max(h0, d.HBs + P * hb) - d.HBs
            rb = min(h1, d.HBs + P * (hb + 1)) - d.HBs
            wl, wr_ = d.hbw[hb]
            mz[ra:rb, wl - wa:wr_ - wa] = m[ra:rb, wl - wa:wr_ - wa]
        m1 = np.ascontiguousarray(
            mz.reshape(2, P, d.Wt).transpose(1, 0, 2)).astype(_bf16)
        d.m_np = np.ascontiguousarray(
            np.broadcast_to(m1[:, None], (P, 2, 2, d.Wt)))

        MT = _conv_matrix(BLUR_RADII[j], KSIZES[j]).T    # MT[src, dst]
        kv = np.zeros((P, 2, d.span), np.float32)
        for k in range(2):
            r0 = d.HBs + P * k
            lo = max(0, -r0)
            hi = min(P, H - r0)
            if hi > lo:
                kv[lo:hi, k, :] = MT[r0 + lo:r0 + hi, h0:h1]
        d.kv_np = np.ascontiguousarray(kv.astype(_bf16))
        kh = np.zeros((P, 2, d.Wr), np.float32)
        for wc in range(2):
            c = d.cstarts[wc]
            kh[:, wc, :] = MT[c:c + P, w0:w1]
        # the second w-chunk overlaps the first: zero duplicated rows
        dup = d.cstarts[0] + P - d.cstarts[1]
        if dup > 0:
            kh[:dup, 1, :] = 0.0
        d.kh_np = np.ascontiguousarray(kh.astype(_bf16))
        drops.append(d)
    return drops, g0, NB, hmin, hmax


def _topo_order(drops):
    """Exact dependency DAG on (block-range x w-range) slice overlap;
    emission order = stable topological levels."""
    def _dep(i, j):
        di, dj_ = drops[i], drops[j]
        if abs(di.B0 - dj_.B0) > 1:
            return False
        ri, wi = (di.wa, di.wb), (di.w0, di.w1)
        rj, wj = (dj_.wa, dj_.wb), (dj_.w0, dj_.w1)
        for (a, b) in ((wi, rj), (ri, wj), (wi, wj)):
            if max(a[0], b[0]) < min(a[1], b[1]):
                return True
        return False

    level = [0] * NUM_DROPS
    preds = {j: [i for i in range(j) if _dep(i, j)] for j in range(NUM_DROPS)}
    for j in range(NUM_DROPS):
        for i in preds[j]:
            level[j] = max(level[j], level[i] + 1)
    # greedy order keeping every dependence >= 2 positions apart, which
    # allows a 3-deep software pipeline (comp trails om by 3 units)
    placed, remaining = [], set(range(NUM_DROPS))
    while remaining:
        cand = [j for j in sorted(remaining, key=lambda j: (level[j], j))
                if all(i not in remaining and placed.index(i) <= len(placed) - 2
                       for i in preds[j])]
        if not cand:
            cand = [j for j in sorted(remaining, key=lambda j: (level[j], j))
                    if all(i not in remaining for i in preds[j])]
        placed.append(cand[0])
        remaining.discard(cand[0])
    order = placed
    pos = {j: p for p, j in enumerate(order)}
    gap2 = all(pos[j] - pos[i] >= 2 for j in range(NUM_DROPS) for i in preds[j])
    skew = 3 if gap2 else 2
    return order, level, skew


def _store_pieces(drops, order, NB):
    """Per block: split [0,W) into up to 3 w-pieces, each tagged with the
    emission position of its LAST writer (-1 = never written)."""
    pieces = {}
    for blk in range(NB):
        last = np.full(W, -1, np.int64)
        for pos, dj in enumerate(order):
            d = drops[dj]
            if d.B0 <= blk <= d.B0 + 1:
                last[d.w0:d.w1] = pos
        runs = []
        ws = 0
        for x in range(1, W + 1):
            if x == W or last[x] != last[ws]:
                runs.append([ws, x, int(last[ws])])
                ws = x
        # merge small runs / cap count; merged run stores after max(pos)
        def _merge_once():
            k = min(range(len(runs)), key=lambda i: runs[i][1] - runs[i][0])
            if k == 0:
                k2 = 1
            elif k == len(runs) - 1:
                k2 = k - 1
            else:
                k2 = k - 1 if (runs[k - 1][1] - runs[k - 1][0]) < (runs[k + 1][1] - runs[k + 1][0]) else k + 1
            a, b = min(k, k2), max(k, k2)
            runs[a] = [runs[a][0], runs[b][1], max(runs[a][2], runs[b][2])]
            del runs[b]
        while len(runs) > 3 or min(r[1] - r[0] for r in runs) < 96:
            _merge_once()
        # even alignment
        for r in runs:
            r[0] &= ~1
        for i in range(len(runs) - 1):
            runs[i][1] = runs[i + 1][0]
        runs[-1][1] = W
        pieces[blk] = [(r[0], r[1], r[2]) for r in runs]
    return pieces


class _Balancer:
    """Greedy static load-balancer across Vector/Scalar/GpSimd with
    HW-measured per-op costs (ns): V sbuf-bf16 TT ~0.62/elem (2x mode),
    V psum-touching 1.1/elem, S copy 1.15/elem, G TT 2.0/elem.
    S (Activation) can only copy; G cannot touch PSUM."""

    def __init__(self, nc):
        self.nc = nc
        self.load = {'V': 0.0, 'S': 0.0, 'G': 0.0}

    def _pick(self, costs):
        eng, c = min(costs, key=lambda ec: self.load[ec[0]] + ec[1])
        self.load[eng] += c
        return eng

    def tt(self, op, out, a, b, fd):
        costs = [('V', fd * 0.95 + 150), ('G', fd * 2.6 + 260)]
        eng = self._pick(costs)
        e = self.nc.vector if eng == 'V' else self.nc.gpsimd
        getattr(e, 'tensor_' + op)(out, a, b)

    def copy(self, out, src, fd):
        # PSUM f32 -> SBUF (V at 1x psum rate, S activation copy)
        eng = self._pick([('V', fd * 1.15 + 200), ('S', fd * 1.1 + 200)])
        if eng == 'V':
            self.nc.vector.tensor_copy(out, src)
        else:
            self.nc.scalar.copy(out=out, in_=src)

    def bsh_mul(self, psb_sl, bshp, m_sl, t2_sl, fd, shape, dt):
        """t2 = m * psb, either via {S|V} psum-copy + {V|G} bf16 mul, or
        V direct mul from PSUM."""
        cV, cS = fd * 1.15 + 200, fd * 1.05 + 200
        mV, mG = fd * 0.65 + 150, fd * 2.6 + 260
        dV = fd * 1.15 + 200
        best, opt = None, None
        for tag, deltas in [('SV', (('S', cS), ('V', mV))),
                            ('SG', (('S', cS), ('G', mG))),
                            ('VG', (('V', cV), ('G', mG))),
                            ('D', (('V', dV),))]:
            tmp = dict(self.load)
            for e, c in deltas:
                tmp[e] += c
            key = (max(tmp.values()), sum(tmp.values()))
            if best is None or key < best:
                best, opt = key, (tag, deltas)
        tag, deltas = opt
        for e, c in deltas:
            self.load[e] += c
        if tag == 'D':
            self.nc.vector.tensor_mul(t2_sl, m_sl, psb_sl)
        else:
            bsh = bshp.tile(shape, dt, tag="Bs")
            bsh_sl = bsh[:, :, :, 0:psb_sl.shape[-1]]
            if tag[0] == 'S':
                self.nc.scalar.copy(out=bsh_sl, in_=psb_sl)
            else:
                self.nc.vector.tensor_copy(bsh_sl, psb_sl)
            e = self.nc.vector if tag[1] == 'V' else self.nc.gpsimd
            e.tensor_mul(t2_sl, m_sl, bsh_sl)


def _build_program(drops, g0, NB, hmin, hmax, order, lvl, skew, pieces):
    from contextlib import ExitStack
    from concourse import bacc, tile, mybir

    f32 = mybir.dt.float32
    bf16 = mybir.dt.bfloat16
    fp8 = mybir.dt.float8e4

    nc = bacc.Bacc("TRN2", target_bir_lowering=False, debug=False,
                   num_devices=N_CORES)

    # input params: per (pair, block-half), always 128 partitions (host
    # zero-pads rows outside [hmin, hmax)); fat DMAs = few triggers
    imgs_d = [[nc.declare_dram_parameter(f"i{g}h{hh}", [P, 2, 2, W], bf16, False)
               for hh in range(2)] for g in range(NG)]
    # output params: per (blk, piece) across ALL channels
    pu = {blk: (max(0, hmin - (g0 + 128 * blk)),
                min(P, hmax - (g0 + 128 * blk))) for blk in range(NB)}
    outs_d = {}
    for blk in range(NB):
        p0, p1 = pu[blk]
        for pi, (ws, we, _pos) in enumerate(pieces[blk]):
            outs_d[(blk, pi)] = nc.declare_dram_parameter(
                f"ob{blk}p{pi}", [p1 - p0, IC, we - ws], bf16, True)

    # drop params batched into 2 chunks by emission order (hot 2 / rest),
    # masks stored once (no jj duplication; ops broadcast via stride-0 AP)
    chunks = [order[0:4], order[4:]]
    kvoffs, khoffs, moffs = {}, {}, {}
    kvlen = [0] * len(chunks)
    khlen = [0] * len(chunks)
    mlen = [0] * len(chunks)
    for ci, ch in enumerate(chunks):
        for dj in ch:
            d = drops[dj]
            kvoffs[dj] = (ci, kvlen[ci])
            khoffs[dj] = (ci, khlen[ci])
            moffs[dj] = (ci, mlen[ci])
            kvlen[ci] += 2 * d.span
            khlen[ci] += 2 * d.Wr
            mlen[ci] += 4 * d.Wt
    negi_d = nc.declare_dram_parameter("negi", [P, P], bf16, False)
    pchunks = []
    for ci, ch in enumerate(chunks):
        pchunks.append((
            nc.declare_dram_parameter(f"mc{ci}", [P, mlen[ci]], bf16, False),
            nc.declare_dram_parameter(f"kvc{ci}", [P, kvlen[ci]], bf16, False),
            nc.declare_dram_parameter(f"khc{ci}", [P, khlen[ci]], bf16, False)))

    bal = _Balancer(nc)

    with tile.TileContext(nc) as tc, ExitStack() as ctx:
        outp = ctx.enter_context(tc.tile_pool(name="out_state", bufs=1))
        out_s = outp.tile([P, IC, NB, W], bf16, name="state", tag="state")
        dp = ctx.enter_context(tc.tile_pool(name="dropin", bufs=1))
        omp = ctx.enter_context(tc.tile_pool(name="omq", bufs=8))
        vtp = ctx.enter_context(tc.tile_pool(name="vts", bufs=8))
        bshp = ctx.enter_context(tc.tile_pool(name="bsh", bufs=8))
        ppa = ctx.enter_context(tc.tile_pool(name="psa", bufs=2, space="PSUM"))
        ppb = ctx.enter_context(tc.tile_pool(name="psb", bufs=2, space="PSUM"))

        # ---- PE warm-up: matmuls on a zeroed tile span the load window
        wt = dp.tile([P, 512], bf16, tag="warm")
        nc.gpsimd.memset(wt[:], 0)
        warm = ppa.tile([P, 2, 2, 256], f32, tag="psa")
        for i in range(14):
            nc.tensor.matmul(warm[:, 0, 0, 0:256], lhsT=wt[:, 0:P],
                             rhs=wt[:, 0:256], start=True, stop=True)
        # pre-zero the vt ring so pass-B stationaries never read NaN garbage
        for i in range(4):
            v0 = vtp.tile([P, 2, 2, 256], bf16, tag="vt", bufs=4)
            (nc.vector if i % 2 else nc.gpsimd).memset(v0[:], 0)

        # ---- loads: params chunk0 on scalar; imgs pair-major on sync so
        # each drop chain starts as its blocks arrive; later chunks follow
        ptiles = []
        for ci, ch in enumerate(chunks):
            ptiles.append((
                dp.tile([P, mlen[ci]], bf16, tag=f"mc{ci}", name=f"mc{ci}"),
                dp.tile([P, kvlen[ci]], bf16, tag=f"kvc{ci}", name=f"kvc{ci}"),
                dp.tile([P, khlen[ci]], bf16, tag=f"khc{ci}", name=f"khc{ci}")))
        # single (sync) queue so bytes arrive strictly in first-use order:
        # hot params (first 4 drops), img halves, then the remaining params
        negi = dp.tile([P, P], bf16, tag="negi", name="negi")
        nc.sync.dma_start(out=negi[:], in_=negi_d.ap()[:])
        for t, pd in zip(ptiles[0], pchunks[0]):
            nc.sync.dma_start(out=t[:], in_=pd.ap()[:])
        for hh in range(2):
            for g in range(NG):
                nc.sync.dma_start(
                    out=out_s[:, 2 * g:2 * g + 2, 2 * hh:2 * hh + 2, :],
                    in_=imgs_d[g][hh].ap()[:])
        for t, pd in zip(ptiles[1], pchunks[1]):
            nc.sync.dma_start(out=t[:], in_=pd.ap()[:])

        # position of each drop in emission order, for store scheduling
        pos_of = {dj: pos for pos, dj in enumerate(order)}
        store_after = {}
        for blk in range(NB):
            for pi, (ws, we, pos) in enumerate(pieces[blk]):
                store_after.setdefault(max(pos, 0), []).append((blk, pi, ws, we))

        # ---- drops: software-pipelined at (drop, pair) granularity so no
        # engine FIFO blocks at its head and PSUM rings (2 bufs each) are
        # recycled only after their reader is emitted.
        #   iteration t: comp(u[t-2]) -> evict+q+passB(u[t-1]) -> om+passA(u[t])
        waves = {}
        for dj in order:
            waves.setdefault(lvl[dj], []).append(dj)

        class _U:
            pass

        def stage1(dj, g):
            u = _U()
            d = drops[dj]
            ci = next(ii for ii, ch in enumerate(chunks) if dj in ch)
            u.d, u.g, u.dj = d, g, dj
            u.i = chunks[ci].index(dj)
            u.mt, u.kvt, u.kht = ptiles[ci]
            _, u.kvo = kvoffs[dj]
            _, u.kho = khoffs[dj]
            _, mo = moffs[dj]
            # mask [P, 2(jj), 2(hb), Wt] materialized (plain strided slices
            # keep the DVE 2x fast path that broadcast APs lose)
            u.m2 = u.mt[:, mo:mo + 4 * d.Wt].rearrange(
                "p (j k w) -> p j k w", j=2, k=2)
            sl = out_s[:, 2 * g:2 * g + 2, d.B0:d.B0 + 2, d.wa:d.wb]
            u.om = omp.tile([P, 2, 2, 256], bf16, tag="om", bufs=4)
            bal.tt('mul', u.om[:, :, :, 0:d.Wt], u.m2, sl, 4 * d.Wt)
            u.psa = ppa.tile([P, 2, 2, 256], f32, tag="psa", bufs=2)
            for wc in range(2):
                coff = d.cstarts[wc] - d.wa
                for jj in range(2):
                    for k in range(2):
                        a, b = d.bandsA[k]
                        nc.tensor.matmul(
                            u.psa[:, jj, wc, a:b],
                            lhsT=u.om[:, jj, k, coff:coff + P],
                            rhs=u.kvt[:, u.kvo + k * d.span + a:u.kvo + k * d.span + b],
                            start=(k == 0), stop=(k == 1))
            return u

        def stage2(u):
            d, g = u.d, u.g
            au, bu = d.wlu - d.w0, d.wru - d.w0
            u.vt = vtp.tile([P, 2, 2, 256], bf16, tag="vt", bufs=4)
            bal.copy(u.vt[:, :, :, d.voff:d.voff + d.span],
                     u.psa[:, :, :, 0:d.span], 4 * d.span)
            u.psb = ppb.tile([P, 2, 2, 256], f32, tag="psb", bufs=2)
            # wc0 streams the full union band (start=True zeroes psb there);
            # wc1 accumulates only its true kh support [c1-p, bu)
            a1 = max(au, d.cstarts[1] - d.p - d.w0)
            for jj in range(2):
                for hb in range(2):
                    nc.tensor.matmul(
                        u.psb[:, jj, hb, au:bu],
                        lhsT=u.vt[:, jj, 0, hb * P:(hb + 1) * P],
                        rhs=u.kht[:, u.kho + au:u.kho + bu],
                        start=True, stop=False)
                    nc.tensor.matmul(
                        u.psb[:, jj, hb, a1:bu],
                        lhsT=u.vt[:, jj, 1, hb * P:(hb + 1) * P],
                        rhs=u.kht[:, u.kho + d.Wr + a1:u.kho + d.Wr + bu],
                        start=False, stop=False)
                    # psb = B - out: -I @ state makes the composite a pure
                    # masked accumulate (q precompute eliminated)
                    nc.tensor.matmul(
                        u.psb[:, jj, hb, au:bu],
                        lhsT=negi[:],
                        rhs=out_s[:, 2 * g + jj, d.B0 + hb, d.wlu:d.wru],
                        start=False, stop=True)

        def stage3(u):
            d, g = u.d, u.g
            au, bu = d.wlu - d.w0, d.wru - d.w0
            widu = bu - au
            t2 = bshp.tile([P, 2, 2, 256], bf16, tag="t2", bufs=4)
            bal.bsh_mul(u.psb[:, :, :, au:bu], bshp,
                        u.m2[:, :, :, d.wlu - d.wa:d.wru - d.wa],
                        t2[:, :, :, 0:widu], 4 * widu, [P, 2, 2, 256], bf16)
            osl = out_s[:, 2 * g:2 * g + 2, d.B0:d.B0 + 2, d.wlu:d.wru]
            bal.tt('add', osl, osl, t2[:, :, :, 0:widu], 4 * widu)
            # store each channel-pair's slice as soon as ITS composite of
            # the piece's last-writer drop lands (shrinks the final drain)
            for (blk, pi, ws, we) in store_after.get(pos_of[u.dj], []):
                p0, p1 = pu[blk]
                nc.sync.dma_start(
                    out=outs_d[(blk, pi)].ap()[:, 2 * g:2 * g + 2],
                    in_=out_s[p0:p1, 2 * g:2 * g + 2, blk, ws:we])

        units = [(dj, g) for dj in order for g in range(NG)]
        ring = []
        for t in range(len(units) + skew):
            if t >= skew and t - skew < len(units):
                stage3(ring[t - skew])
            if t >= 1 and t - 1 < len(units):
                stage2(ring[t - 1])
            if t < len(units):
                ring.append(stage1(*units[t]))
    nc.compile()
    print("balancer loads (us):",
          {k: round(v / 1000, 1) for k, v in bal.load.items()})
    return nc


_CACHE = {}


def _get_program(positions, radius):
    key = (np.asarray(positions, np.float32).tobytes(),
           np.asarray(radius, np.float32).tobytes())
    if key not in _CACHE:
        drops, g0, NB, hmin, hmax = _drop_meta(positions, radius)
        order, level, skew = _topo_order(drops)
        print("emission order:", order, "levels:", level, "skew:", skew)
        pieces = _store_pieces(drops, order, NB)
        nc = _build_program(drops, g0, NB, hmin, hmax, order, level, skew, pieces)
        _CACHE[key] = (nc, drops, g0, NB, hmin, hmax, order, pieces)
    return _CACHE[key]


def kernel(img, positions, radius, _want_trace=False, **_kw):
    from concourse.bass_utils import run_bass_kernel_spmd
    img = np.asarray(img, np.float32)
    assert img.shape == (B_TOTAL, C, H, W)
    nc, drops, g0, NB, hmin, hmax, order, pieces = _get_program(positions, radius)

    # pack rows [g0, g0+NB*128) to [p, pair(2), w] per (core, pair, blk), bf16,
    # zero-padded outside [hmin, hmax)
    rows_lo, rows_hi = hmin, hmax
    imgb = np.zeros((N_CORES, IC, NB * P, W), _bf16)
    src = img.reshape(N_CORES, IC, H, W)
    imgb[:, :, rows_lo - g0:rows_hi - g0, :] = src[:, :, rows_lo:rows_hi, :].astype(_bf16)
    packed = np.ascontiguousarray(
        imgb.reshape(N_CORES, IC, NB, P, W).transpose(0, 3, 1, 2, 4))

    chunks = [order[0:4], order[4:]]
    base = {}
    for ci, ch in enumerate(chunks):
        base[f"mc{ci}"] = np.ascontiguousarray(np.concatenate(
            [drops[dj].m_np.reshape(P, -1) for dj in ch], axis=1))
        base[f"kvc{ci}"] = np.ascontiguousarray(np.concatenate(
            [drops[dj].kv_np.reshape(P, -1) for dj in ch], axis=1))
        base[f"khc{ci}"] = np.ascontiguousarray(np.concatenate(
            [drops[dj].kh_np.reshape(P, -1) for dj in ch], axis=1))
    base["negi"] = np.ascontiguousarray((-np.eye(P)).astype(_bf16))
    in_maps = []
    for i in range(N_CORES):
        mp = dict(base)
        for g in range(NG):
            for hh in range(2):
                mp[f"i{g}h{hh}"] = np.ascontiguousarray(
                    packed[i][:, 2 * g:2 * g + 2, 2 * hh:2 * hh + 2, :])
        in_maps.append(mp)
    res = run_bass_kernel_spmd(nc, in_maps, core_ids=list(range(N_CORES)),
                               trace=_want_trace)
    out = img.copy()
    pu = {blk: (max(0, hmin - (g0 + 128 * blk)),
                min(P, hmax - (g0 + 128 * blk))) for blk in range(NB)}
    for i in range(N_CORES):
        oc = out.reshape(N_CORES, IC, H, W)
        for blk in range(NB):
            p0, p1 = pu[blk]
            r0 = g0 + 128 * blk + p0
            for pi, (ws, we, _pos) in enumerate(pieces[blk]):
                blkres = res.results[i][f"ob{blk}p{pi}"]
                # [Pu, IC, wlen] -> rows r0..r0+Pu
                oc[i, :, r0:r0 + (p1 - p0), ws:we] = \
                    blkres.transpose(1, 0, 2).astype(np.float32)
    if _want_trace:
        return out, res
    return out


# revision 49
# speedup vs baseline: 1.0370x; 1.0370x over previous
"""Trainium2 Bass kernel for nn_Condensation: 10 sequential masked-Gaussian-blur
composites over a [16,3,768,768] image, data-parallel over 8 NeuronCores.

v22 strategy (per core, 2 images = 6 image-channels):
  - Row-offset block grid (delta chosen so EVERY drop's mask support fits in
    exactly 2 h-blocks of 128 rows): ~25% less elementwise/matmul work than a
    0-based grid and no false inter-drop deps from block padding.
  - Drops emitted in exact topological order of the true spatial-overlap DAG
    (non-overlapping drops commute): 3 levels of 4/4/2 drops.
  - Single continuous software pipeline over all 30 (drop, channel-pair)
    units at skew 3 (comp[u-3] <- evict/q/passB[u-1] <- om/passA[u]), using
    an emission order that keeps every drop dependence >= 2 positions apart
    (falls back to skew 2 if impossible); PSUM rings (2 psa + 2 psb bufs =
    8 banks) recycle only after their reader is emitted.
  - State in SBUF as bf16 [128, 6, NB, 768]; only rows any drop touches are
    loaded/stored (partial-partition edge blocks, zero-padded fat loads: one
    DMA per (pair, block-half)).
  - Masks materialized per (jj, h-block) as plain strided slices (stride-0
    broadcast APs lose the DVE fast path); no +-p w-margins anywhere
    (sources outside [w0,w1) are mask-zero). All loads ride ONE queue in
    first-use order: params of first 4 drops, image halves, rest.
  - Separable blur: two banded bf16 matmul passes on TensorE (f32 PSUM);
    pass B streams the full union band on the first w-chunk (start=True
    zeroes psb), the true kh support on the second, then a third matmul
    accumulates -I @ state so psb = blur - out ARRIVES pre-subtracted.
  - Composite is therefore just t2 = m*psb and an in-place out += t2 (the
    old q = out - om precompute and its slow strided reads are gone --
    ~25% of the Vector/GpSimd elementwise work moved onto idle TensorE
    capacity, exactly, in f32 PSUM). Per-op greedy balance across
    Vector/Scalar/GpSimd with HW-trace-calibrated contended costs.
  - Stores split per (block, w-piece, channel-pair), each issued as soon as
    its piece's last-writer drop composites that pair, so output DMA drains
    throughout instead of at the end.
"""
import numpy as np
import ml_dtypes

NUM_DROPS = 10
MIN_R, MAX_R = 60.0, 80.0
BETA = 1.8
BLUR_RADII = [11.3535, 17.9381, 5.7966, 10.8586, 5.5301, 15.9075, 12.3225, 13.4871, 6.6639, 9.5413]


def _ksize(r):
    k = int(2 * r) + 1
    return k + 1 if k % 2 == 0 else k


KSIZES = [_ksize(r) for r in BLUR_RADII]
H = W = 768
B_TOTAL, C = 16, 3
N_CORES = 8
B_LOC = B_TOTAL // N_CORES          # 2 images per core
IC = B_LOC * C                      # 6 image-channels per core
NG = IC // 2                        # 3 pairs of image-channels
P = 128
EPS = 5e-3                          # mask support threshold (error-validated)

_bf16 = ml_dtypes.bfloat16
_fp8 = ml_dtypes.float8_e4m3fn


def _conv_matrix(sigma, ksize, n=768):
    """n x n matrix Kmat with blur_1d(x) = Kmat @ x, matching the reference
    (correlation with normalized gaussian, 'reflect' padding)."""
    half = (ksize - 1) * 0.5
    xs = np.linspace(-half, half, ksize)
    pdf = np.exp(-0.5 * (xs / np.float64(sigma)) ** 2)
    k1 = (pdf / pdf.sum()).astype(np.float32).astype(np.float64)
    pad = ksize // 2
    Kmat = np.zeros((n, n), dtype=np.float64)
    idx = np.arange(n)[:, None] + np.arange(ksize)[None, :] - pad
    idx = np.abs(idx)
    idx = np.where(idx >= n, 2 * n - 2 - idx, idx)
    np.add.at(Kmat, (np.repeat(np.arange(n), ksize), idx.ravel()),
              np.tile(k1, n))
    return Kmat.astype(np.float32)


class _Drop:
    pass


def _drop_meta(positions, radius):
    """Host-side per-drop geometry + tensors (shared across cores) on the
    row-offset block grid."""
    pos = np.clip(np.asarray(positions, np.float32), -1.0, 1.0)
    rad = np.clip(np.asarray(radius, np.float32), MIN_R, MAX_R)
    s = float(np.sqrt((-np.log(EPS)) ** (1.0 / BETA)))
    s2 = s * s

    geo = []
    for j in range(NUM_DROPS):
        x0 = (pos[j, 0] + 1.0) / 2.0 * W
        y0 = (pos[j, 1] + 1.0) / 2.0 * H
        wr = rad[j]
        hr = wr * np.float32(0.8)
        p = KSIZES[j] // 2
        h0 = max(0, int(np.floor(y0 - s * hr))) & ~1
        h1 = min(H, (int(np.ceil(y0 + s * hr)) + 2) & ~1)
        w0 = max(0, int(np.floor(x0 - s * wr))) & ~1
        w1 = min(W, (int(np.ceil(x0 + s * wr)) + 2) & ~1)
        geo.append([h0, h1, w0, w1, p, float(x0), float(y0), float(wr), float(hr)])

    # pick an even grid offset so every drop spans exactly 2 blocks
    delta = None
    for dd_ in range(0, 128, 2):
        if all(((g[0] - dd_) % 128) + (g[1] - g[0]) <= 256 for g in geo):
            delta = dd_
            break
    assert delta is not None, "no 2-block grid offset exists"
    hmin = min(g[0] for g in geo)
    hmax = max(g[1] for g in geo)
    g0 = hmin - ((hmin - delta) % 128)
    NB = -((g0 - hmax) // 128)

    drops = []
    for j in range(NUM_DROPS):
        h0, h1, w0, w1, p, x0, y0, wr, hr = geo[j]
        d = _Drop()
        d.j, d.p = j, p
        d.B0 = (h0 - g0) // 128
        d.HBs = g0 + 128 * d.B0
        assert h1 - d.HBs <= 256 and d.B0 + 2 <= NB
        # cap w so Wt <= 256 (two overlapping 128-col chunks)
        wcap = 256 - 2 * p - 2
        while w1 - w0 > wcap:
            if x0 - w0 > w1 - x0:
                w0 += 2
            else:
                w1 -= 2
        d.h0, d.h1, d.w0, d.w1 = h0, h1, w0, w1
        d.span = h1 - h0
        d.Wr = w1 - w0
        d.voff = h0 - d.HBs
        # no margin: sources outside [w0,w1) are mask-zero, so the
        # horizontal pass can contract over [w0,w1) only
        wa, wb = w0, w1
        d.wa, d.wb = wa, wb
        d.Wt = wb - wa
        assert d.Wt <= 256 and d.span <= 256
        d.WBn = (d.Wt + P - 1) // P
        assert d.WBn == 2
        d.cstarts = [wa, wb - P]

        # pass A bands per k-block: output h' range (relative to h0)
        d.bandsA = []
        for k in range(2):
            a = max(0, d.HBs + P * k - p - h0)
            b = min(d.span, d.HBs + P * (k + 1) + p - h0)
            d.bandsA.append((a, b))

        # per h-block composite w-range [wl, wr) from the ellipse extent
        d.hbw = []
        for hb in range(2):
            ra = max(h0, d.HBs + P * hb)
            rb = min(h1, d.HBs + P * (hb + 1))
            if ra - 1 < y0 < rb:
                dh = 0.0
            else:
                dh = min(abs(ra - y0), abs(rb - 1 - y0))
            half = wr * np.sqrt(max(0.0, s2 - (dh / hr) ** 2))
            wl = max(w0, (int(np.floor(x0 - half)) - 2) & ~1)
            wr_ = min(w1, (int(np.ceil(x0 + half)) + 4) & ~1)
            wr_ = max(wr_, wl + 2)
            d.hbw.append((wl, wr_))
        # union composite window across both h-blocks (mask is zero outside
        # each block's own [wl, wr), so fused ops over the union are exact)
        d.wlu = min(wl for wl, _ in d.hbw)
        d.wru = max(wr_ for _, wr_ in d.hbw)

        # mask over [2 blocks of 128 rows] x [wa:wb], zero outside support
        rows = (d.HBs + np.arange(2 * P, dtype=np.int64)).astype(np.float32)
        dd = (rows[:, None] - y0) ** 2 / hr ** 2 + \
             (np.arange(wa, wb, dtype=np.float32)[None, :] - x0) ** 2 / wr ** 2
        m = np.clip(np.exp(-(dd.astype(np.float32) ** np.float32(BETA)) + np.float32(1e-10)), 0.0, 1.0)
        m = np.where(dd <= np.float32(s2), m, 0.0).astype(np.float32)
        mz = np.zeros_like(m)
        for hb in range(2):
            ra = max(h0, d.HBs + P * hb) - d.HBs
            rb = min(h1, d.HBs + P * (hb + 1)) - d.HBs
            wl, wr_ = d.hbw[hb]
            mz[ra:rb, wl - wa:wr_ - wa] = m[ra:rb, wl - wa:wr_ - wa]
        m1 = np.ascontiguousarray(
            mz.reshape(2, P, d.Wt).transpose(1, 0, 2)).astype(_bf16)
        d.m_np = np.ascontiguousarray(
            np.broadcast_to(m1[:, None], (P, 2, 2, d.Wt)))

        MT = _conv_matrix(BLUR_RADII[j], KSIZES[j]).T    # MT[src, dst]
        kv = np.zeros((P, 2, d.span), np.float32)
        for k in range(2):
            r0 = d.HBs + P * k
            lo = max(0, -r0)
            hi = min(P, H - r0)
            if hi > lo:
                kv[lo:hi, k, :] = MT[r0 + lo:r0 + hi, h0:h1]
        d.kv_np = np.ascontiguousarray(kv.astype(_bf16))
        kh = np.zeros((P, 2, d.Wr), np.float32)
        for wc in range(2):
            c = d.cstarts[wc]
            kh[:, wc, :] = MT[c:c + P, w0:w1]
        # the second w-chunk overlaps the first: zero duplicated rows
        dup = d.cstarts[0] + P - d.cstarts[1]
        if dup > 0:
            kh[:dup, 1, :] = 0.0
        d.kh_np = np.ascontiguousarray(kh.astype(_bf16))
        drops.append(d)
    return drops, g0, NB, hmin, hmax


def _topo_order(drops):
    """Exact dependency DAG on (block-range x w-range) slice overlap;
    emission order = stable topological levels."""
    def _dep(i, j):
        di, dj_ = drops[i], drops[j]
        if abs(di.B0 - dj_.B0) > 1:
            return False
        ri, wi = (di.wa, di.wb), (di.w0, di.w1)
        rj, wj = (dj_.wa, dj_.wb), (dj_.w0, dj_.w1)
        for (a, b) in ((wi, rj), (ri, wj), (wi, wj)):
            if max(a[0], b[0]) < min(a[1], b[1]):
                return True
        return False

    level = [0] * NUM_DROPS
    preds = {j: [i for i in range(j) if _dep(i, j)] for j in range(NUM_DROPS)}
    for j in range(NUM_DROPS):
        for i in preds[j]:
            level[j] = max(level[j], level[i] + 1)
    # greedy order keeping every dependence >= 2 positions apart, which
    # allows a 3-deep software pipeline (comp trails om by 3 units)
    placed, remaining = [], set(range(NUM_DROPS))
    while remaining:
        cand = [j for j in sorted(remaining, key=lambda j: (level[j], j))
                if all(i not in remaining and placed.index(i) <= len(placed) - 2
                       for i in preds[j])]
        if not cand:
            cand = [j for j in sorted(remaining, key=lambda j: (level[j], j))
                    if all(i not in remaining for i in preds[j])]
        placed.append(cand[0])
        remaining.discard(cand[0])
    order = placed
    pos = {j: p for p, j in enumerate(order)}
    gap2 = all(pos[j] - pos[i] >= 2 for j in range(NUM_DROPS) for i in preds[j])
    skew = 3 if gap2 else 2
    return order, level, skew


def _store_pieces(drops, order, NB):
    """Per block: split [0,W) into up to 3 w-pieces, each tagged with the
    emission position of its LAST writer (-1 = never written)."""
    pieces = {}
    for blk in range(NB):
        last = np.full(W, -1, np.int64)
        for pos, dj in enumerate(order):
            d = drops[dj]
            if d.B0 <= blk <= d.B0 + 1:
                last[d.w0:d.w1] = pos
        runs = []
        ws = 0
        for x in range(1, W + 1):
            if x == W or last[x] != last[ws]:
                runs.append([ws, x, int(last[ws])])
                ws = x
        # merge small runs / cap count; merged run stores after max(pos)
        def _merge_once():
            k = min(range(len(runs)), key=lambda i: runs[i][1] - runs[i][0])
            if k == 0:
                k2 = 1
            elif k == len(runs) - 1:
                k2 = k - 1
            else:
                k2 = k - 1 if (runs[k - 1][1] - runs[k - 1][0]) < (runs[k + 1][1] - runs[k + 1][0]) else k + 1
            a, b = min(k, k2), max(k, k2)
            runs[a] = [runs[a][0], runs[b][1], max(runs[a][2], runs[b][2])]
            del runs[b]
        while len(runs) > 3 or min(r[1] - r[0] for r in runs) < 96:
            _merge_once()
        # even alignment
        for r in runs:
            r[0] &= ~1
        for i in range(len(runs) - 1):
            runs[i][1] = runs[i + 1][0]
        runs[-1][1] = W
        pieces[blk] = [(r[0], r[1], r[2]) for r in runs]
    return pieces


class _Balancer:
    """Greedy static load-balancer across Vector/Scalar/GpSimd with
    HW-measured per-op costs (ns): V sbuf-bf16 TT ~0.62/elem (2x mode),
    V psum-touching 1.1/elem, S copy 1.15/elem, G TT 2.0/elem.
    S (Activation) can only copy; G cannot touch PSUM."""

    def __init__(self, nc):
        self.nc = nc
        self.load = {'V': 0.0, 'S': 0.0, 'G': 0.0}

    def _pick(self, costs):
        eng, c = min(costs, key=lambda ec: self.load[ec[0]] + ec[1])
        self.load[eng] += c
        return eng

    def tt(self, op, out, a, b, fd):
        costs = [('V', fd * 0.95 + 150), ('G', fd * 2.6 + 260)]
        eng = self._pick(costs)
        e = self.nc.vector if eng == 'V' else self.nc.gpsimd
        getattr(e, 'tensor_' + op)(out, a, b)

    def copy(self, out, src, fd):
        # PSUM f32 -> SBUF (V at 1x psum rate, S activation copy)
        eng = self._pick([('V', fd * 1.15 + 200), ('S', fd * 1.1 + 200)])
        if eng == 'V':
            self.nc.vector.tensor_copy(out, src)
        else:
            self.nc.scalar.copy(out=out, in_=src)

    def bsh_mul(self, psb_sl, bshp, m_sl, t2_sl, fd, shape, dt):
        """t2 = m * psb, either via {S|V} psum-copy + {V|G} bf16 mul, or
        V direct mul from PSUM."""
        cV, cS = fd * 1.15 + 200, fd * 1.05 + 200
        mV, mG = fd * 0.65 + 150, fd * 2.6 + 260
        dV = fd * 1.15 + 200
        best, opt = None, None
        for tag, deltas in [('SV', (('S', cS), ('V', mV))),
                            ('SG', (('S', cS), ('G', mG))),
                            ('VG', (('V', cV), ('G', mG))),
                            ('D', (('V', dV),))]:
            tmp = dict(self.load)
            for e, c in deltas:
                tmp[e] += c
            key = (max(tmp.values()), sum(tmp.values()))
            if best is None or key < best:
                best, opt = key, (tag, deltas)
        tag, deltas = opt
        for e, c in deltas:
            self.load[e] += c
        if tag == 'D':
            self.nc.vector.tensor_mul(t2_sl, m_sl, psb_sl)
        else:
            bsh = bshp.tile(shape, dt, tag="Bs")
            bsh_sl = bsh[:, :, :, 0:psb_sl.shape[-1]]
            if tag[0] == 'S':
                self.nc.scalar.copy(out=bsh_sl, in_=psb_sl)
            else:
                self.nc.vector.tensor_copy(bsh_sl, psb_sl)
            e = self.nc.vector if tag[1] == 'V' else self.nc.gpsimd
            e.tensor_mul(t2_sl, m_sl, bsh_sl)


def _build_program(drops, g0, NB, hmin, hmax, order, lvl, skew, pieces):
    from contextlib import ExitStack
    from concourse import bacc, tile, mybir

    f32 = mybir.dt.float32
    bf16 = mybir.dt.bfloat16
    fp8 = mybir.dt.float8e4

    nc = bacc.Bacc("TRN2", target_bir_lowering=False, debug=False,
                   num_devices=N_CORES)

    # input params: per (pair, block-half), always 128 partitions (host
    # zero-pads rows outside [hmin, hmax)); fat DMAs = few triggers
    imgs_d = [[nc.declare_dram_parameter(f"i{g}h{hh}", [P, 2, 2, W], bf16, False)
               for hh in range(2)] for g in range(NG)]
    # output params: per (blk, piece) across ALL channels
    pu = {blk: (max(0, hmin - (g0 + 128 * blk)),
                min(P, hmax - (g0 + 128 * blk))) for blk in range(NB)}
    outs_d = {}
    for blk in range(NB):
        p0, p1 = pu[blk]
        for pi, (ws, we, _pos) in enumerate(pieces[blk]):
            outs_d[(blk, pi)] = nc.declare_dram_parameter(
                f"ob{blk}p{pi}", [p1 - p0, IC, we - ws], bf16, True)

    # drop params batched into 2 chunks by emission order (hot 2 / rest),
    # masks stored once (no jj duplication; ops broadcast via stride-0 AP)
    chunks = [order[0:2], order[2:4], order[4:]]
    kvoffs, khoffs, moffs = {}, {}, {}
    kvlen = [0] * len(chunks)
    khlen = [0] * len(chunks)
    mlen = [0] * len(chunks)
    for ci, ch in enumerate(chunks):
        for dj in ch:
            d = drops[dj]
            kvoffs[dj] = (ci, kvlen[ci])
            khoffs[dj] = (ci, khlen[ci])
            moffs[dj] = (ci, mlen[ci])
            kvlen[ci] += 2 * d.span
            khlen[ci] += 2 * d.Wr
            mlen[ci] += 4 * d.Wt
    negi_d = nc.declare_dram_parameter("negi", [P, P], bf16, False)
    pchunks = []
    for ci, ch in enumerate(chunks):
        pchunks.append((
            nc.declare_dram_parameter(f"mc{ci}", [P, mlen[ci]], bf16, False),
            nc.declare_dram_parameter(f"kvc{ci}", [P, kvlen[ci]], bf16, False),
            nc.declare_dram_parameter(f"khc{ci}", [P, khlen[ci]], bf16, False)))

    bal = _Balancer(nc)

    with tile.TileContext(nc) as tc, ExitStack() as ctx:
        outp = ctx.enter_context(tc.tile_pool(name="out_state", bufs=1))
        out_s = outp.tile([P, IC, NB, W], bf16, name="state", tag="state")
        dp = ctx.enter_context(tc.tile_pool(name="dropin", bufs=1))
        omp = ctx.enter_context(tc.tile_pool(name="omq", bufs=8))
        vtp = ctx.enter_context(tc.tile_pool(name="vts", bufs=8))
        bshp = ctx.enter_context(tc.tile_pool(name="bsh", bufs=8))
        ppa = ctx.enter_context(tc.tile_pool(name="psa", bufs=2, space="PSUM"))
        ppb = ctx.enter_context(tc.tile_pool(name="psb", bufs=2, space="PSUM"))

        # ---- PE warm-up: matmuls on a zeroed tile span the load window
        wt = dp.tile([P, 512], bf16, tag="warm")
        nc.gpsimd.memset(wt[:], 0)
        warm = ppa.tile([P, 2, 2, 256], f32, tag="psa")
        for i in range(14):
            nc.tensor.matmul(warm[:, 0, 0, 0:256], lhsT=wt[:, 0:P],
                             rhs=wt[:, 0:256], start=True, stop=True)
        # pre-zero the vt ring so pass-B stationaries never read NaN garbage
        for i in range(4):
            v0 = vtp.tile([P, 2, 2, 256], bf16, tag="vt", bufs=4)
            (nc.vector if i % 2 else nc.gpsimd).memset(v0[:], 0)

        # ---- loads: params chunk0 on scalar; imgs pair-major on sync so
        # each drop chain starts as its blocks arrive; later chunks follow
        ptiles = []
        for ci, ch in enumerate(chunks):
            ptiles.append((
                dp.tile([P, mlen[ci]], bf16, tag=f"mc{ci}", name=f"mc{ci}"),
                dp.tile([P, kvlen[ci]], bf16, tag=f"kvc{ci}", name=f"kvc{ci}"),
                dp.tile([P, khlen[ci]], bf16, tag=f"khc{ci}", name=f"khc{ci}")))
        # single (sync) queue so bytes arrive strictly in first-use order:
        # hot params (first 4 drops), img halves, then the remaining params
        negi = dp.tile([P, P], bf16, tag="negi", name="negi")
        nc.sync.dma_start(out=negi[:], in_=negi_d.ap()[:])
        for t, pd in zip(ptiles[0], pchunks[0]):
            nc.sync.dma_start(out=t[:], in_=pd.ap()[:])
        for hh in range(2):
            for g in range(NG):
                nc.sync.dma_start(
                    out=out_s[:, 2 * g:2 * g + 2, 2 * hh:2 * hh + 2, :],
                    in_=imgs_d[g][hh].ap()[:])
        for ci in (1, 2):
            for t, pd in zip(ptiles[ci], pchunks[ci]):
                nc.sync.dma_start(out=t[:], in_=pd.ap()[:])

        # position of each drop in emission order, for store scheduling
        pos_of = {dj: pos for pos, dj in enumerate(order)}
        store_after = {}
        for blk in range(NB):
            for pi, (ws, we, pos) in enumerate(pieces[blk]):
                store_after.setdefault(max(pos, 0), []).append((blk, pi, ws, we))

        # ---- drops: software-pipelined at (drop, pair) granularity so no
        # engine FIFO blocks at its head and PSUM rings (2 bufs each) are
        # recycled only after their reader is emitted.
        #   iteration t: comp(u[t-2]) -> evict+q+passB(u[t-1]) -> om+passA(u[t])
        waves = {}
        for dj in order:
            waves.setdefault(lvl[dj], []).append(dj)

        class _U:
            pass

        def stage1(dj, g):
            u = _U()
            d = drops[dj]
            ci = next(ii for ii, ch in enumerate(chunks) if dj in ch)
            u.d, u.g, u.dj = d, g, dj
            u.i = chunks[ci].index(dj)
            u.mt, u.kvt, u.kht = ptiles[ci]
            _, u.kvo = kvoffs[dj]
            _, u.kho = khoffs[dj]
            _, mo = moffs[dj]
            # mask [P, 2(jj), 2(hb), Wt] materialized (plain strided slices
            # keep the DVE 2x fast path that broadcast APs lose)
            u.m2 = u.mt[:, mo:mo + 4 * d.Wt].rearrange(
                "p (j k w) -> p j k w", j=2, k=2)
            sl = out_s[:, 2 * g:2 * g + 2, d.B0:d.B0 + 2, d.wa:d.wb]
            u.om = omp.tile([P, 2, 2, 256], bf16, tag="om", bufs=4)
            bal.tt('mul', u.om[:, :, :, 0:d.Wt], u.m2, sl, 4 * d.Wt)
            u.psa = ppa.tile([P, 2, 2, 256], f32, tag="psa", bufs=2)
            for wc in range(2):
                coff = d.cstarts[wc] - d.wa
                for jj in range(2):
                    for k in range(2):
                        a, b = d.bandsA[k]
                        nc.tensor.matmul(
                            u.psa[:, jj, wc, a:b],
                            lhsT=u.om[:, jj, k, coff:coff + P],
                            rhs=u.kvt[:, u.kvo + k * d.span + a:u.kvo + k * d.span + b],
                            start=(k == 0), stop=(k == 1))
            return u

        def stage2(u):
            d, g = u.d, u.g
            au, bu = d.wlu - d.w0, d.wru - d.w0
            u.vt = vtp.tile([P, 2, 2, 256], bf16, tag="vt", bufs=4)
            bal.copy(u.vt[:, :, :, d.voff:d.voff + d.span],
                     u.psa[:, :, :, 0:d.span], 4 * d.span)
            u.psb = ppb.tile([P, 2, 2, 256], f32, tag="psb", bufs=2)
            # wc0 streams the full union band (start=True zeroes psb there);
            # wc1 accumulates only its true kh support [c1-p, bu)
            a1 = max(au, d.cstarts[1] - d.p - d.w0)
            for jj in range(2):
                for hb in range(2):
                    nc.tensor.matmul(
                        u.psb[:, jj, hb, au:bu],
                        lhsT=u.vt[:, jj, 0, hb * P:(hb + 1) * P],
                        rhs=u.kht[:, u.kho + au:u.kho + bu],
                        start=True, stop=False)
                    nc.tensor.matmul(
                        u.psb[:, jj, hb, a1:bu],
                        lhsT=u.vt[:, jj, 1, hb * P:(hb + 1) * P],
                        rhs=u.kht[:, u.kho + d.Wr + a1:u.kho + d.Wr + bu],
                        start=False, stop=False)
                    # psb = B - out: -I @ state makes the composite a pure
                    # masked accumulate (q precompute eliminated)
                    nc.tensor.matmul(
                        u.psb[:, jj, hb, au:bu],
                        lhsT=negi[:],
                        rhs=out_s[:, 2 * g + jj, d.B0 + hb, d.wlu:d.wru],
                        start=False, stop=True)

        def stage3(u):
            d, g = u.d, u.g
            au, bu = d.wlu - d.w0, d.wru - d.w0
            widu = bu - au
            t2 = bshp.tile([P, 2, 2, 256], bf16, tag="t2", bufs=4)
            bal.bsh_mul(u.psb[:, :, :, au:bu], bshp,
                        u.m2[:, :, :, d.wlu - d.wa:d.wru - d.wa],
                        t2[:, :, :, 0:widu], 4 * widu, [P, 2, 2, 256], bf16)
            osl = out_s[:, 2 * g:2 * g + 2, d.B0:d.B0 + 2, d.wlu:d.wru]
            bal.tt('add', osl, osl, t2[:, :, :, 0:widu], 4 * widu)
            # store each channel-pair's slice as soon as ITS composite of
            # the piece's last-writer drop lands (shrinks the final drain)
            for (blk, pi, ws, we) in store_after.get(pos_of[u.dj], []):
                p0, p1 = pu[blk]
                nc.sync.dma_start(
                    out=outs_d[(blk, pi)].ap()[:, 2 * g:2 * g + 2],
                    in_=out_s[p0:p1, 2 * g:2 * g + 2, blk, ws:we])

        units = [(dj, g) for dj in order for g in range(NG)]
        ring = []
        for t in range(len(units) + skew):
            if t >= skew and t - skew < len(units):
                stage3(ring[t - skew])
            if t >= 1 and t - 1 < len(units):
                stage2(ring[t - 1])
            if t < len(units):
                ring.append(stage1(*units[t]))
    nc.compile()
    print("balancer loads (us):",
          {k: round(v / 1000, 1) for k, v in bal.load.items()})
    return nc


_CACHE = {}


def _get_program(positions, radius):
    key = (np.asarray(positions, np.float32).tobytes(),
           np.asarray(radius, np.float32).tobytes())
    if key not in _CACHE:
        drops, g0, NB, hmin, hmax = _drop_meta(positions, radius)
        order, level, skew = _topo_order(drops)
        print("emission order:", order, "levels:", level, "skew:", skew)
        pieces = _store_pieces(drops, order, NB)
        nc = _build_program(drops, g0, NB, hmin, hmax, order, level, skew, pieces)
        _CACHE[key] = (nc, drops, g0, NB, hmin, hmax, order, pieces)
    return _CACHE[key]


def kernel(img, positions, radius, _want_trace=False, **_kw):
    from concourse.bass_utils import run_bass_kernel_spmd
    img = np.asarray(img, np.float32)
    assert img.shape == (B_TOTAL, C, H, W)
    nc, drops, g0, NB, hmin, hmax, order, pieces = _get_program(positions, radius)

    # pack rows [g0, g0+NB*128) to [p, pair(2), w] per (core, pair, blk), bf16,
    # zero-padded outside [hmin, hmax)
    rows_lo, rows_hi = hmin, hmax
    imgb = np.zeros((N_CORES, IC, NB * P, W), _bf16)
    src = img.reshape(N_CORES, IC, H, W)
    imgb[:, :, rows_lo - g0:rows_hi - g0, :] = src[:, :, rows_lo:rows_hi, :].astype(_bf16)
    packed = np.ascontiguousarray(
        imgb.reshape(N_CORES, IC, NB, P, W).transpose(0, 3, 1, 2, 4))

    chunks = [order[0:2], order[2:4], order[4:]]
    base = {}
    for ci, ch in enumerate(chunks):
        base[f"mc{ci}"] = np.ascontiguousarray(np.concatenate(
            [drops[dj].m_np.reshape(P, -1) for dj in ch], axis=1))
        base[f"kvc{ci}"] = np.ascontiguousarray(np.concatenate(
            [drops[dj].kv_np.reshape(P, -1) for dj in ch], axis=1))
        base[f"khc{ci}"] = np.ascontiguousarray(np.concatenate(
            [drops[dj].kh_np.reshape(P, -1) for dj in ch], axis=1))
    base["negi"] = np.ascontiguousarray((-np.eye(P)).astype(_bf16))
    in_maps = []
    for i in range(N_CORES):
        mp = dict(base)
        for g in range(NG):
            for hh in range(2):
                mp[f"i{g}h{hh}"] = np.ascontiguousarray(
                    packed[i][:, 2 * g:2 * g + 2, 2 * hh:2 * hh + 2, :])
        in_maps.append(mp)
    res = run_bass_kernel_spmd(nc, in_maps, core_ids=list(range(N_CORES)),
                               trace=_want_trace)
    out = img.copy()
    pu = {blk: (max(0, hmin - (g0 + 128 * blk)),
                min(P, hmax - (g0 + 128 * blk))) for blk in range(NB)}
    for i in range(N_CORES):
        oc = out.reshape(N_CORES, IC, H, W)
        for blk in range(NB):
            p0, p1 = pu[blk]
            r0 = g0 + 128 * blk + p0
            for pi, (ws, we, _pos) in enumerate(pieces[blk]):
                blkres = res.results[i][f"ob{blk}p{pi}"]
                # [Pu, IC, wlen] -> rows r0..r0+Pu
                oc[i, :, r0:r0 + (p1 - p0), ws:we] = \
                    blkres.transpose(1, 0, 2).astype(np.float32)
    if _want_trace:
        return out, res
    return out


# revision 50
# speedup vs baseline: 1.0492x; 1.0117x over previous
"""Trainium2 Bass kernel for nn_Condensation: 10 sequential masked-Gaussian-blur
composites over a [16,3,768,768] image, data-parallel over 8 NeuronCores.

v22 strategy (per core, 2 images = 6 image-channels):
  - Row-offset block grid (delta chosen so EVERY drop's mask support fits in
    exactly 2 h-blocks of 128 rows): ~25% less elementwise/matmul work than a
    0-based grid and no false inter-drop deps from block padding.
  - Drops emitted in exact topological order of the true spatial-overlap DAG
    (non-overlapping drops commute): 3 levels of 4/4/2 drops.
  - Single continuous software pipeline over all 30 (drop, channel-pair)
    units at skew 3 (comp[u-3] <- evict/q/passB[u-1] <- om/passA[u]), using
    an emission order that keeps every drop dependence >= 2 positions apart
    (falls back to skew 2 if impossible); PSUM rings (2 psa + 2 psb bufs =
    8 banks) recycle only after their reader is emitted.
  - State in SBUF as bf16 [128, 6, NB, 768]; only rows any drop touches are
    loaded/stored (partial-partition edge blocks, zero-padded fat loads: one
    DMA per (pair, block-half)).
  - Masks materialized per (jj, h-block) as plain strided slices (stride-0
    broadcast APs lose the DVE fast path); no +-p w-margins anywhere
    (sources outside [w0,w1) are mask-zero). All loads ride ONE queue in
    first-use order: params of first 4 drops, image halves, rest.
  - Separable blur: two banded bf16 matmul passes on TensorE (f32 PSUM);
    pass B streams the full union band on the first w-chunk (start=True
    zeroes psb), the true kh support on the second, then a third matmul
    accumulates -I @ state so psb = blur - out ARRIVES pre-subtracted.
  - Composite is therefore just t2 = m*psb and an in-place out += t2 (the
    old q = out - om precompute and its slow strided reads are gone --
    ~25% of the Vector/GpSimd elementwise work moved onto idle TensorE
    capacity, exactly, in f32 PSUM). Per-op greedy balance across
    Vector/Scalar/GpSimd with HW-trace-calibrated contended costs.
  - Stores split per (block, w-piece, channel-pair), each issued as soon as
    its piece's last-writer drop composites that pair, so output DMA drains
    throughout instead of at the end.
"""
import numpy as np
import ml_dtypes

NUM_DROPS = 10
MIN_R, MAX_R = 60.0, 80.0
BETA = 1.8
BLUR_RADII = [11.3535, 17.9381, 5.7966, 10.8586, 5.5301, 15.9075, 12.3225, 13.4871, 6.6639, 9.5413]


def _ksize(r):
    k = int(2 * r) + 1
    return k + 1 if k % 2 == 0 else k


KSIZES = [_ksize(r) for r in BLUR_RADII]
H = W = 768
B_TOTAL, C = 16, 3
N_CORES = 8
B_LOC = B_TOTAL // N_CORES          # 2 images per core
IC = B_LOC * C                      # 6 image-channels per core
NG = IC // 2                        # 3 pairs of image-channels
P = 128
EPS = 5e-3                          # mask support threshold (error-validated)

_bf16 = ml_dtypes.bfloat16
_fp8 = ml_dtypes.float8_e4m3fn


def _conv_matrix(sigma, ksize, n=768):
    """n x n matrix Kmat with blur_1d(x) = Kmat @ x, matching the reference
    (correlation with normalized gaussian, 'reflect' padding)."""
    half = (ksize - 1) * 0.5
    xs = np.linspace(-half, half, ksize)
    pdf = np.exp(-0.5 * (xs / np.float64(sigma)) ** 2)
    k1 = (pdf / pdf.sum()).astype(np.float32).astype(np.float64)
    pad = ksize // 2
    Kmat = np.zeros((n, n), dtype=np.float64)
    idx = np.arange(n)[:, None] + np.arange(ksize)[None, :] - pad
    idx = np.abs(idx)
    idx = np.where(idx >= n, 2 * n - 2 - idx, idx)
    np.add.at(Kmat, (np.repeat(np.arange(n), ksize), idx.ravel()),
              np.tile(k1, n))
    return Kmat.astype(np.float32)


class _Drop:
    pass


def _drop_meta(positions, radius):
    """Host-side per-drop geometry + tensors (shared across cores) on the
    row-offset block grid."""
    pos = np.clip(np.asarray(positions, np.float32), -1.0, 1.0)
    rad = np.clip(np.asarray(radius, np.float32), MIN_R, MAX_R)
    s = float(np.sqrt((-np.log(EPS)) ** (1.0 / BETA)))
    s2 = s * s

    geo = []
    for j in range(NUM_DROPS):
        x0 = (pos[j, 0] + 1.0) / 2.0 * W
        y0 = (pos[j, 1] + 1.0) / 2.0 * H
        wr = rad[j]
        hr = wr * np.float32(0.8)
        p = KSIZES[j] // 2
        h0 = max(0, int(np.floor(y0 - s * hr))) & ~1
        h1 = min(H, (int(np.ceil(y0 + s * hr)) + 2) & ~1)
        w0 = max(0, int(np.floor(x0 - s * wr))) & ~1
        w1 = min(W, (int(np.ceil(x0 + s * wr)) + 2) & ~1)
        geo.append([h0, h1, w0, w1, p, float(x0), float(y0), float(wr), float(hr)])

    # pick an even grid offset so every drop spans exactly 2 blocks
    delta = None
    for dd_ in range(0, 128, 2):
        if all(((g[0] - dd_) % 128) + (g[1] - g[0]) <= 256 for g in geo):
            delta = dd_
            break
    assert delta is not None, "no 2-block grid offset exists"
    hmin = min(g[0] for g in geo)
    hmax = max(g[1] for g in geo)
    g0 = hmin - ((hmin - delta) % 128)
    NB = -((g0 - hmax) // 128)

    drops = []
    for j in range(NUM_DROPS):
        h0, h1, w0, w1, p, x0, y0, wr, hr = geo[j]
        d = _Drop()
        d.j, d.p = j, p
        d.B0 = (h0 - g0) // 128
        d.HBs = g0 + 128 * d.B0
        assert h1 - d.HBs <= 256 and d.B0 + 2 <= NB
        # cap w so Wt <= 256 (two overlapping 128-col chunks)
        wcap = 256 - 2 * p - 2
        while w1 - w0 > wcap:
            if x0 - w0 > w1 - x0:
                w0 += 2
            else:
                w1 -= 2
        d.h0, d.h1, d.w0, d.w1 = h0, h1, w0, w1
        d.span = h1 - h0
        d.Wr = w1 - w0
        d.voff = h0 - d.HBs
        # no margin: sources outside [w0,w1) are mask-zero, so the
        # horizontal pass can contract over [w0,w1) only
        wa, wb = w0, w1
        d.wa, d.wb = wa, wb
        d.Wt = wb - wa
        assert d.Wt <= 256 and d.span <= 256
        d.WBn = (d.Wt + P - 1) // P
        assert d.WBn == 2
        d.cstarts = [wa, wb - P]

        # pass A bands per k-block: output h' range (relative to h0)
        d.bandsA = []
        for k in range(2):
            a = max(0, d.HBs + P * k - p - h0)
            b = min(d.span, d.HBs + P * (k + 1) + p - h0)
            d.bandsA.append((a, b))

        # per h-block composite w-range [wl, wr) from the ellipse extent
        d.hbw = []
        for hb in range(2):
            ra = max(h0, d.HBs + P * hb)
            rb = min(h1, d.HBs + P * (hb + 1))
            if ra - 1 < y0 < rb:
                dh = 0.0
            else:
                dh = min(abs(ra - y0), abs(rb - 1 - y0))
            half = wr * np.sqrt(max(0.0, s2 - (dh / hr) ** 2))
            wl = max(w0, (int(np.floor(x0 - half)) - 2) & ~1)
            wr_ = min(w1, (int(np.ceil(x0 + half)) + 4) & ~1)
            wr_ = max(wr_, wl + 2)
            d.hbw.append((wl, wr_))
        # union composite window across both h-blocks (mask is zero outside
        # each block's own [wl, wr), so fused ops over the union are exact)
        d.wlu = min(wl for wl, _ in d.hbw)
        d.wru = max(wr_ for _, wr_ in d.hbw)

        # mask over [2 blocks of 128 rows] x [wa:wb], zero outside support
        rows = (d.HBs + np.arange(2 * P, dtype=np.int64)).astype(np.float32)
        dd = (rows[:, None] - y0) ** 2 / hr ** 2 + \
             (np.arange(wa, wb, dtype=np.float32)[None, :] - x0) ** 2 / wr ** 2
        m = np.clip(np.exp(-(dd.astype(np.float32) ** np.float32(BETA)) + np.float32(1e-10)), 0.0, 1.0)
        m = np.where(dd <= np.float32(s2), m, 0.0).astype(np.float32)
        mz = np.zeros_like(m)
        for hb in range(2):
            ra = max(h0, d.HBs + P * hb) - d.HBs
            rb = min(h1, d.HBs + P * (hb + 1)) - d.HBs
            wl, wr_ = d.hbw[hb]
            mz[ra:rb, wl - wa:wr_ - wa] = m[ra:rb, wl - wa:wr_ - wa]
        m1 = np.ascontiguousarray(
            mz.reshape(2, P, d.Wt).transpose(1, 0, 2)).astype(_bf16)
        d.m_np = np.ascontiguousarray(
            np.broadcast_to(m1[:, None], (P, 2, 2, d.Wt)))

        MT = _conv_matrix(BLUR_RADII[j], KSIZES[j]).T    # MT[src, dst]
        kv = np.zeros((P, 2, d.span), np.float32)
        for k in range(2):
            r0 = d.HBs + P * k
            lo = max(0, -r0)
            hi = min(P, H - r0)
            if hi > lo:
                kv[lo:hi, k, :] = MT[r0 + lo:r0 + hi, h0:h1]
        d.kv_np = np.ascontiguousarray(kv.astype(_bf16))
        kh = np.zeros((P, 2, d.Wr), np.float32)
        for wc in range(2):
            c = d.cstarts[wc]
            kh[:, wc, :] = MT[c:c + P, w0:w1]
        # the second w-chunk overlaps the first: zero duplicated rows
        dup = d.cstarts[0] + P - d.cstarts[1]
        if dup > 0:
            kh[:dup, 1, :] = 0.0
        d.kh_np = np.ascontiguousarray(kh.astype(_bf16))
        drops.append(d)
    return drops, g0, NB, hmin, hmax


def _topo_order(drops):
    """Exact dependency DAG on (block-range x w-range) slice overlap;
    emission order = stable topological levels."""
    def _dep(i, j):
        di, dj_ = drops[i], drops[j]
        if abs(di.B0 - dj_.B0) > 1:
            return False
        ri, wi = (di.wa, di.wb), (di.w0, di.w1)
        rj, wj = (dj_.wa, dj_.wb), (dj_.w0, dj_.w1)
        for (a, b) in ((wi, rj), (ri, wj), (wi, wj)):
            if max(a[0], b[0]) < min(a[1], b[1]):
                return True
        return False

    level = [0] * NUM_DROPS
    preds = {j: [i for i in range(j) if _dep(i, j)] for j in range(NUM_DROPS)}
    for j in range(NUM_DROPS):
        for i in preds[j]:
            level[j] = max(level[j], level[i] + 1)
    # greedy order keeping every dependence >= 2 positions apart, which
    # allows a 3-deep software pipeline (comp trails om by 3 units)
    placed, remaining = [], set(range(NUM_DROPS))
    while remaining:
        cand = [j for j in sorted(remaining, key=lambda j: (level[j], j))
                if all(i not in remaining and placed.index(i) <= len(placed) - 2
                       for i in preds[j])]
        if not cand:
            cand = [j for j in sorted(remaining, key=lambda j: (level[j], j))
                    if all(i not in remaining for i in preds[j])]
        placed.append(cand[0])
        remaining.discard(cand[0])
    order = placed
    pos = {j: p for p, j in enumerate(order)}
    gap2 = all(pos[j] - pos[i] >= 2 for j in range(NUM_DROPS) for i in preds[j])
    skew = 3 if gap2 else 2
    return order, level, skew


def _store_pieces(drops, order, NB):
    """Per block: split [0,W) into up to 3 w-pieces, each tagged with the
    emission position of its LAST writer (-1 = never written)."""
    pieces = {}
    for blk in range(NB):
        last = np.full(W, -1, np.int64)
        for pos, dj in enumerate(order):
            d = drops[dj]
            if d.B0 <= blk <= d.B0 + 1:
                last[d.w0:d.w1] = pos
        runs = []
        ws = 0
        for x in range(1, W + 1):
            if x == W or last[x] != last[ws]:
                runs.append([ws, x, int(last[ws])])
                ws = x
        # merge small runs / cap count; merged run stores after max(pos)
        def _merge_once():
            k = min(range(len(runs)), key=lambda i: runs[i][1] - runs[i][0])
            if k == 0:
                k2 = 1
            elif k == len(runs) - 1:
                k2 = k - 1
            else:
                k2 = k - 1 if (runs[k - 1][1] - runs[k - 1][0]) < (runs[k + 1][1] - runs[k + 1][0]) else k + 1
            a, b = min(k, k2), max(k, k2)
            runs[a] = [runs[a][0], runs[b][1], max(runs[a][2], runs[b][2])]
            del runs[b]
        while len(runs) > 3 or min(r[1] - r[0] for r in runs) < 96:
            _merge_once()
        # even alignment
        for r in runs:
            r[0] &= ~1
        for i in range(len(runs) - 1):
            runs[i][1] = runs[i + 1][0]
        runs[-1][1] = W
        pieces[blk] = [(r[0], r[1], r[2]) for r in runs]
    return pieces


class _Balancer:
    """Greedy static load-balancer across Vector/Scalar/GpSimd with
    HW-measured per-op costs (ns): V sbuf-bf16 TT ~0.62/elem (2x mode),
    V psum-touching 1.1/elem, S copy 1.15/elem, G TT 2.0/elem.
    S (Activation) can only copy; G cannot touch PSUM."""

    def __init__(self, nc):
        self.nc = nc
        self.load = {'V': 0.0, 'S': 0.0, 'G': 0.0}

    def _pick(self, costs):
        eng, c = min(costs, key=lambda ec: self.load[ec[0]] + ec[1])
        self.load[eng] += c
        return eng

    def tt(self, op, out, a, b, fd):
        costs = [('V', fd * 0.95 + 150), ('G', fd * 2.6 + 260)]
        eng = self._pick(costs)
        e = self.nc.vector if eng == 'V' else self.nc.gpsimd
        getattr(e, 'tensor_' + op)(out, a, b)

    def copy(self, out, src, fd):
        # PSUM f32 -> SBUF (V at 1x psum rate, S activation copy)
        eng = self._pick([('V', fd * 1.15 + 200), ('S', fd * 1.1 + 200)])
        if eng == 'V':
            self.nc.vector.tensor_copy(out, src)
        else:
            self.nc.scalar.copy(out=out, in_=src)

    def bsh_mul(self, psb_sl, bshp, m_sl, t2_sl, fd, shape, dt):
        """t2 = m * psb, either via {S|V} psum-copy + {V|G} bf16 mul, or
        V direct mul from PSUM."""
        cV, cS = fd * 1.15 + 200, fd * 1.05 + 200
        mV, mG = fd * 0.65 + 150, fd * 2.6 + 260
        dV = fd * 1.15 + 200
        best, opt = None, None
        for tag, deltas in [('SV', (('S', cS), ('V', mV))),
                            ('SG', (('S', cS), ('G', mG))),
                            ('VG', (('V', cV), ('G', mG))),
                            ('D', (('V', dV),))]:
            tmp = dict(self.load)
            for e, c in deltas:
                tmp[e] += c
            key = (max(tmp.values()), sum(tmp.values()))
            if best is None or key < best:
                best, opt = key, (tag, deltas)
        tag, deltas = opt
        for e, c in deltas:
            self.load[e] += c
        if tag == 'D':
            self.nc.vector.tensor_mul(t2_sl, m_sl, psb_sl)
        else:
            bsh = bshp.tile(shape, dt, tag="Bs")
            bsh_sl = bsh[:, :, :, 0:psb_sl.shape[-1]]
            if tag[0] == 'S':
                self.nc.scalar.copy(out=bsh_sl, in_=psb_sl)
            else:
                self.nc.vector.tensor_copy(bsh_sl, psb_sl)
            e = self.nc.vector if tag[1] == 'V' else self.nc.gpsimd
            e.tensor_mul(t2_sl, m_sl, bsh_sl)


def _build_program(drops, g0, NB, hmin, hmax, order, lvl, skew, pieces):
    from contextlib import ExitStack
    from concourse import bacc, tile, mybir

    f32 = mybir.dt.float32
    bf16 = mybir.dt.bfloat16
    fp8 = mybir.dt.float8e4

    nc = bacc.Bacc("TRN2", target_bir_lowering=False, debug=False,
                   num_devices=N_CORES)

    # input params: per (pair, block-half), always 128 partitions (host
    # zero-pads rows outside [hmin, hmax)); fat DMAs = few triggers
    imgs_d = [[nc.declare_dram_parameter(f"i{g}h{hh}", [P, 2, 2, W], bf16, False)
               for hh in range(2)] for g in range(NG)]
    # output params: per (blk, piece) across ALL channels
    pu = {blk: (max(0, hmin - (g0 + 128 * blk)),
                min(P, hmax - (g0 + 128 * blk))) for blk in range(NB)}
    outs_d = {}
    for blk in range(NB):
        p0, p1 = pu[blk]
        for pi, (ws, we, _pos) in enumerate(pieces[blk]):
            outs_d[(blk, pi)] = nc.declare_dram_parameter(
                f"ob{blk}p{pi}", [p1 - p0, IC, we - ws], bf16, True)

    # drop params batched into 2 chunks by emission order (hot 2 / rest),
    # masks stored once (no jj duplication; ops broadcast via stride-0 AP)
    chunks = [order[0:2], order[2:4], order[4:]]
    kvoffs, khoffs, moffs = {}, {}, {}
    kvlen = [0] * len(chunks)
    khlen = [0] * len(chunks)
    mlen = [0] * len(chunks)
    for ci, ch in enumerate(chunks):
        for dj in ch:
            d = drops[dj]
            kvoffs[dj] = (ci, kvlen[ci])
            khoffs[dj] = (ci, khlen[ci])
            moffs[dj] = (ci, mlen[ci])
            kvlen[ci] += 2 * d.span
            khlen[ci] += 2 * d.Wr
            mlen[ci] += 4 * d.Wt
    negi_d = nc.declare_dram_parameter("negi", [P, P], bf16, False)
    pchunks = []
    for ci, ch in enumerate(chunks):
        pchunks.append((
            nc.declare_dram_parameter(f"mc{ci}", [P, mlen[ci]], bf16, False),
            nc.declare_dram_parameter(f"kvc{ci}", [P, kvlen[ci]], bf16, False),
            nc.declare_dram_parameter(f"khc{ci}", [P, khlen[ci]], bf16, False)))

    bal = _Balancer(nc)

    with tile.TileContext(nc) as tc, ExitStack() as ctx:
        outp = ctx.enter_context(tc.tile_pool(name="out_state", bufs=1))
        out_s = outp.tile([P, IC, NB, W], bf16, name="state", tag="state")
        dp = ctx.enter_context(tc.tile_pool(name="dropin", bufs=1))
        omp = ctx.enter_context(tc.tile_pool(name="omq", bufs=8))
        vtp = ctx.enter_context(tc.tile_pool(name="vts", bufs=8))
        bshp = ctx.enter_context(tc.tile_pool(name="bsh", bufs=8))
        ppa = ctx.enter_context(tc.tile_pool(name="psa", bufs=2, space="PSUM"))
        ppb = ctx.enter_context(tc.tile_pool(name="psb", bufs=2, space="PSUM"))

        # ---- PE warm-up: matmuls on a zeroed tile span the load window
        wt = dp.tile([P, 512], bf16, tag="warm")
        nc.gpsimd.memset(wt[:], 0)
        warm = ppa.tile([P, 2, 2, 256], f32, tag="psa")
        for i in range(26):
            nc.tensor.matmul(warm[:, 0, 0, 0:256], lhsT=wt[:, 0:P],
                             rhs=wt[:, 0:256], start=True, stop=True)
        # pre-zero the vt ring so pass-B stationaries never read NaN garbage
        for i in range(4):
            v0 = vtp.tile([P, 2, 2, 256], bf16, tag="vt", bufs=4)
            (nc.vector if i % 2 else nc.gpsimd).memset(v0[:], 0)

        # ---- loads: params chunk0 on scalar; imgs pair-major on sync so
        # each drop chain starts as its blocks arrive; later chunks follow
        ptiles = []
        for ci, ch in enumerate(chunks):
            ptiles.append((
                dp.tile([P, mlen[ci]], bf16, tag=f"mc{ci}", name=f"mc{ci}"),
                dp.tile([P, kvlen[ci]], bf16, tag=f"kvc{ci}", name=f"kvc{ci}"),
                dp.tile([P, khlen[ci]], bf16, tag=f"khc{ci}", name=f"khc{ci}")))
        # single (sync) queue so bytes arrive strictly in first-use order:
        # hot params (first 4 drops), img halves, then the remaining params
        negi = dp.tile([P, P], bf16, tag="negi", name="negi")
        nc.sync.dma_start(out=negi[:], in_=negi_d.ap()[:])
        for t, pd in zip(ptiles[0], pchunks[0]):
            nc.sync.dma_start(out=t[:], in_=pd.ap()[:])
        for hh in range(2):
            for g in range(NG):
                nc.sync.dma_start(
                    out=out_s[:, 2 * g:2 * g + 2, 2 * hh:2 * hh + 2, :],
                    in_=imgs_d[g][hh].ap()[:])
        for ci in (1, 2):
            for t, pd in zip(ptiles[ci], pchunks[ci]):
                nc.sync.dma_start(out=t[:], in_=pd.ap()[:])

        # position of each drop in emission order, for store scheduling
        pos_of = {dj: pos for pos, dj in enumerate(order)}
        store_after = {}
        for blk in range(NB):
            for pi, (ws, we, pos) in enumerate(pieces[blk]):
                store_after.setdefault(max(pos, 0), []).append((blk, pi, ws, we))

        # ---- drops: software-pipelined at (drop, pair) granularity so no
        # engine FIFO blocks at its head and PSUM rings (2 bufs each) are
        # recycled only after their reader is emitted.
        #   iteration t: comp(u[t-2]) -> evict+q+passB(u[t-1]) -> om+passA(u[t])
        waves = {}
        for dj in order:
            waves.setdefault(lvl[dj], []).append(dj)

        class _U:
            pass

        def stage1(dj, g):
            u = _U()
            d = drops[dj]
            ci = next(ii for ii, ch in enumerate(chunks) if dj in ch)
            u.d, u.g, u.dj = d, g, dj
            u.i = chunks[ci].index(dj)
            u.mt, u.kvt, u.kht = ptiles[ci]
            _, u.kvo = kvoffs[dj]
            _, u.kho = khoffs[dj]
            _, mo = moffs[dj]
            # mask [P, 2(jj), 2(hb), Wt] materialized (plain strided slices
            # keep the DVE 2x fast path that broadcast APs lose)
            u.m2 = u.mt[:, mo:mo + 4 * d.Wt].rearrange(
                "p (j k w) -> p j k w", j=2, k=2)
            sl = out_s[:, 2 * g:2 * g + 2, d.B0:d.B0 + 2, d.wa:d.wb]
            u.om = omp.tile([P, 2, 2, 256], bf16, tag="om", bufs=4)
            bal.tt('mul', u.om[:, :, :, 0:d.Wt], u.m2, sl, 4 * d.Wt)
            u.psa = ppa.tile([P, 2, 2, 256], f32, tag="psa", bufs=2)
            for wc in range(2):
                coff = d.cstarts[wc] - d.wa
                for jj in range(2):
                    for k in range(2):
                        a, b = d.bandsA[k]
                        nc.tensor.matmul(
                            u.psa[:, jj, wc, a:b],
                            lhsT=u.om[:, jj, k, coff:coff + P],
                            rhs=u.kvt[:, u.kvo + k * d.span + a:u.kvo + k * d.span + b],
                            start=(k == 0), stop=(k == 1))
            return u

        def stage2(u):
            d, g = u.d, u.g
            au, bu = d.wlu - d.w0, d.wru - d.w0
            u.vt = vtp.tile([P, 2, 2, 256], bf16, tag="vt", bufs=4)
            bal.copy(u.vt[:, :, :, d.voff:d.voff + d.span],
                     u.psa[:, :, :, 0:d.span], 4 * d.span)
            u.psb = ppb.tile([P, 2, 2, 256], f32, tag="psb", bufs=2)
            # wc0 streams the full union band (start=True zeroes psb there);
            # wc1 accumulates only its true kh support [c1-p, bu)
            a1 = max(au, d.cstarts[1] - d.p - d.w0)
            for jj in range(2):
                for hb in range(2):
                    nc.tensor.matmul(
                        u.psb[:, jj, hb, au:bu],
                        lhsT=u.vt[:, jj, 0, hb * P:(hb + 1) * P],
                        rhs=u.kht[:, u.kho + au:u.kho + bu],
                        start=True, stop=False)
                    nc.tensor.matmul(
                        u.psb[:, jj, hb, a1:bu],
                        lhsT=u.vt[:, jj, 1, hb * P:(hb + 1) * P],
                        rhs=u.kht[:, u.kho + d.Wr + a1:u.kho + d.Wr + bu],
                        start=False, stop=False)
                    # psb = B - out: -I @ state makes the composite a pure
                    # masked accumulate (q precompute eliminated)
                    nc.tensor.matmul(
                        u.psb[:, jj, hb, au:bu],
                        lhsT=negi[:],
                        rhs=out_s[:, 2 * g + jj, d.B0 + hb, d.wlu:d.wru],
                        start=False, stop=True)

        def stage3(u):
            d, g = u.d, u.g
            au, bu = d.wlu - d.w0, d.wru - d.w0
            widu = bu - au
            t2 = bshp.tile([P, 2, 2, 256], bf16, tag="t2", bufs=4)
            bal.bsh_mul(u.psb[:, :, :, au:bu], bshp,
                        u.m2[:, :, :, d.wlu - d.wa:d.wru - d.wa],
                        t2[:, :, :, 0:widu], 4 * widu, [P, 2, 2, 256], bf16)
            osl = out_s[:, 2 * g:2 * g + 2, d.B0:d.B0 + 2, d.wlu:d.wru]
            bal.tt('add', osl, osl, t2[:, :, :, 0:widu], 4 * widu)
            # store each channel-pair's slice as soon as ITS composite of
            # the piece's last-writer drop lands (shrinks the final drain)
            for (blk, pi, ws, we) in store_after.get(pos_of[u.dj], []):
                p0, p1 = pu[blk]
                nc.sync.dma_start(
                    out=outs_d[(blk, pi)].ap()[:, 2 * g:2 * g + 2],
                    in_=out_s[p0:p1, 2 * g:2 * g + 2, blk, ws:we])

        units = [(dj, g) for dj in order for g in range(NG)]
        ring = []
        for t in range(len(units) + skew):
            if t >= skew and t - skew < len(units):
                stage3(ring[t - skew])
            if t >= 1 and t - 1 < len(units):
                stage2(ring[t - 1])
            if t < len(units):
                ring.append(stage1(*units[t]))
    nc.compile()
    print("balancer loads (us):",
          {k: round(v / 1000, 1) for k, v in bal.load.items()})
    return nc


_CACHE = {}


def _get_program(positions, radius):
    key = (np.asarray(positions, np.float32).tobytes(),
           np.asarray(radius, np.float32).tobytes())
    if key not in _CACHE:
        drops, g0, NB, hmin, hmax = _drop_meta(positions, radius)
        order, level, skew = _topo_order(drops)
        print("emission order:", order, "levels:", level, "skew:", skew)
        pieces = _store_pieces(drops, order, NB)
        nc = _build_program(drops, g0, NB, hmin, hmax, order, level, skew, pieces)
        _CACHE[key] = (nc, drops, g0, NB, hmin, hmax, order, pieces)
    return _CACHE[key]


def kernel(img, positions, radius, _want_trace=False, **_kw):
    from concourse.bass_utils import run_bass_kernel_spmd
    img = np.asarray(img, np.float32)
    assert img.shape == (B_TOTAL, C, H, W)
    nc, drops, g0, NB, hmin, hmax, order, pieces = _get_program(positions, radius)

    # pack rows [g0, g0+NB*128) to [p, pair(2), w] per (core, pair, blk), bf16,
    # zero-padded outside [hmin, hmax)
    rows_lo, rows_hi = hmin, hmax
    imgb = np.zeros((N_CORES, IC, NB * P, W), _bf16)
    src = img.reshape(N_CORES, IC, H, W)
    imgb[:, :, rows_lo - g0:rows_hi - g0, :] = src[:, :, rows_lo:rows_hi, :].astype(_bf16)
    packed = np.ascontiguousarray(
        imgb.reshape(N_CORES, IC, NB, P, W).transpose(0, 3, 1, 2, 4))

    chunks = [order[0:2], order[2:4], order[4:]]
    base = {}
    for ci, ch in enumerate(chunks):
        base[f"mc{ci}"] = np.ascontiguousarray(np.concatenate(
            [drops[dj].m_np.reshape(P, -1) for dj in ch], axis=1))
        base[f"kvc{ci}"] = np.ascontiguousarray(np.concatenate(
            [drops[dj].kv_np.reshape(P, -1) for dj in ch], axis=1))
        base[f"khc{ci}"] = np.ascontiguousarray(np.concatenate(
            [drops[dj].kh_np.reshape(P, -1) for dj in ch], axis=1))
    base["negi"] = np.ascontiguousarray((-np.eye(P)).astype(_bf16))
    in_maps = []
    for i in range(N_CORES):
        mp = dict(base)
        for g in range(NG):
            for hh in range(2):
                mp[f"i{g}h{hh}"] = np.ascontiguousarray(
                    packed[i][:, 2 * g:2 * g + 2, 2 * hh:2 * hh + 2, :])
        in_maps.append(mp)
    res = run_bass_kernel_spmd(nc, in_maps, core_ids=list(range(N_CORES)),
                               trace=_want_trace)
    out = img.copy()
    pu = {blk: (max(0, hmin - (g0 + 128 * blk)),
                min(P, hmax - (g0 + 128 * blk))) for blk in range(NB)}
    for i in range(N_CORES):
        oc = out.reshape(N_CORES, IC, H, W)
        for blk in range(NB):
            p0, p1 = pu[blk]
            r0 = g0 + 128 * blk + p0
            for pi, (ws, we, _pos) in enumerate(pieces[blk]):
                blkres = res.results[i][f"ob{blk}p{pi}"]
                # [Pu, IC, wlen] -> rows r0..r0+Pu
                oc[i, :, r0:r0 + (p1 - p0), ws:we] = \
                    blkres.transpose(1, 0, 2).astype(np.float32)
    if _want_trace:
        return out, res
    return out


# revision 51
# speedup vs baseline: 1.0613x; 1.0116x over previous
"""Trainium2 Bass kernel for nn_Condensation: 10 sequential masked-Gaussian-blur
composites over a [16,3,768,768] image, data-parallel over 8 NeuronCores.

v22 strategy (per core, 2 images = 6 image-channels):
  - Row-offset block grid (delta chosen so EVERY drop's mask support fits in
    exactly 2 h-blocks of 128 rows): ~25% less elementwise/matmul work than a
    0-based grid and no false inter-drop deps from block padding.
  - Drops emitted in exact topological order of the true spatial-overlap DAG
    (non-overlapping drops commute): 3 levels of 4/4/2 drops.
  - Single continuous software pipeline over all 30 (drop, channel-pair)
    units at skew 3 (comp[u-3] <- evict/q/passB[u-1] <- om/passA[u]), using
    an emission order that keeps every drop dependence >= 2 positions apart
    (falls back to skew 2 if impossible); PSUM rings (2 psa + 2 psb bufs =
    8 banks) recycle only after their reader is emitted.
  - State in SBUF as bf16 [128, 6, NB, 768]; only rows any drop touches are
    loaded/stored (partial-partition edge blocks, zero-padded fat loads: one
    DMA per (pair, block-half)).
  - Masks materialized per (jj, h-block) as plain strided slices (stride-0
    broadcast APs lose the DVE fast path); no +-p w-margins anywhere
    (sources outside [w0,w1) are mask-zero). All loads ride ONE queue in
    first-use order: params of first 4 drops, image halves, rest.
  - Separable blur: two banded bf16 matmul passes on TensorE (f32 PSUM);
    pass B streams the full union band on the first w-chunk (start=True
    zeroes psb), the true kh support on the second, then a third matmul
    accumulates -I @ state so psb = blur - out ARRIVES pre-subtracted.
  - Composite is therefore just t2 = m*psb and an in-place out += t2 (the
    old q = out - om precompute and its slow strided reads are gone --
    ~25% of the Vector/GpSimd elementwise work moved onto idle TensorE
    capacity, exactly, in f32 PSUM). Per-op greedy balance across
    Vector/Scalar/GpSimd with HW-trace-calibrated contended costs.
  - Stores split per (block, w-piece, channel-pair), each issued as soon as
    its piece's last-writer drop composites that pair, so output DMA drains
    throughout instead of at the end.
"""
import numpy as np
import ml_dtypes

NUM_DROPS = 10
MIN_R, MAX_R = 60.0, 80.0
BETA = 1.8
BLUR_RADII = [11.3535, 17.9381, 5.7966, 10.8586, 5.5301, 15.9075, 12.3225, 13.4871, 6.6639, 9.5413]


def _ksize(r):
    k = int(2 * r) + 1
    return k + 1 if k % 2 == 0 else k


KSIZES = [_ksize(r) for r in BLUR_RADII]
H = W = 768
B_TOTAL, C = 16, 3
N_CORES = 8
B_LOC = B_TOTAL // N_CORES          # 2 images per core
IC = B_LOC * C                      # 6 image-channels per core
NG = IC // 2                        # 3 pairs of image-channels
P = 128
EPS = 5e-3                          # mask support threshold (error-validated)

_bf16 = ml_dtypes.bfloat16
_fp8 = ml_dtypes.float8_e4m3fn


def _conv_matrix(sigma, ksize, n=768):
    """n x n matrix Kmat with blur_1d(x) = Kmat @ x, matching the reference
    (correlation with normalized gaussian, 'reflect' padding)."""
    half = (ksize - 1) * 0.5
    xs = np.linspace(-half, half, ksize)
    pdf = np.exp(-0.5 * (xs / np.float64(sigma)) ** 2)
    k1 = (pdf / pdf.sum()).astype(np.float32).astype(np.float64)
    pad = ksize // 2
    Kmat = np.zeros((n, n), dtype=np.float64)
    idx = np.arange(n)[:, None] + np.arange(ksize)[None, :] - pad
    idx = np.abs(idx)
    idx = np.where(idx >= n, 2 * n - 2 - idx, idx)
    np.add.at(Kmat, (np.repeat(np.arange(n), ksize), idx.ravel()),
              np.tile(k1, n))
    return Kmat.astype(np.float32)


class _Drop:
    pass


def _drop_meta(positions, radius):
    """Host-side per-drop geometry + tensors (shared across cores) on the
    row-offset block grid."""
    pos = np.clip(np.asarray(positions, np.float32), -1.0, 1.0)
    rad = np.clip(np.asarray(radius, np.float32), MIN_R, MAX_R)
    s = float(np.sqrt((-np.log(EPS)) ** (1.0 / BETA)))
    s2 = s * s

    geo = []
    for j in range(NUM_DROPS):
        x0 = (pos[j, 0] + 1.0) / 2.0 * W
        y0 = (pos[j, 1] + 1.0) / 2.0 * H
        wr = rad[j]
        hr = wr * np.float32(0.8)
        p = KSIZES[j] // 2
        h0 = max(0, int(np.floor(y0 - s * hr))) & ~1
        h1 = min(H, (int(np.ceil(y0 + s * hr)) + 2) & ~1)
        w0 = max(0, int(np.floor(x0 - s * wr))) & ~1
        w1 = min(W, (int(np.ceil(x0 + s * wr)) + 2) & ~1)
        geo.append([h0, h1, w0, w1, p, float(x0), float(y0), float(wr), float(hr)])

    # pick an even grid offset so every drop spans exactly 2 blocks
    delta = None
    for dd_ in range(0, 128, 2):
        if all(((g[0] - dd_) % 128) + (g[1] - g[0]) <= 256 for g in geo):
            delta = dd_
            break
    assert delta is not None, "no 2-block grid offset exists"
    hmin = min(g[0] for g in geo)
    hmax = max(g[1] for g in geo)
    g0 = hmin - ((hmin - delta) % 128)
    NB = -((g0 - hmax) // 128)

    drops = []
    for j in range(NUM_DROPS):
        h0, h1, w0, w1, p, x0, y0, wr, hr = geo[j]
        d = _Drop()
        d.j, d.p = j, p
        d.B0 = (h0 - g0) // 128
        d.HBs = g0 + 128 * d.B0
        assert h1 - d.HBs <= 256 and d.B0 + 2 <= NB
        # cap w so Wt <= 256 (two overlapping 128-col chunks)
        wcap = 256 - 2 * p - 2
        while w1 - w0 > wcap:
            if x0 - w0 > w1 - x0:
                w0 += 2
            else:
                w1 -= 2
        d.h0, d.h1, d.w0, d.w1 = h0, h1, w0, w1
        d.span = h1 - h0
        d.Wr = w1 - w0
        d.voff = h0 - d.HBs
        # no margin: sources outside [w0,w1) are mask-zero, so the
        # horizontal pass can contract over [w0,w1) only
        wa, wb = w0, w1
        d.wa, d.wb = wa, wb
        d.Wt = wb - wa
        assert d.Wt <= 256 and d.span <= 256
        d.WBn = (d.Wt + P - 1) // P
        assert d.WBn == 2
        d.cstarts = [wa, wb - P]

        # pass A bands per k-block: output h' range (relative to h0)
        d.bandsA = []
        for k in range(2):
            a = max(0, d.HBs + P * k - p - h0)
            b = min(d.span, d.HBs + P * (k + 1) + p - h0)
            d.bandsA.append((a, b))

        # per h-block composite w-range [wl, wr) from the ellipse extent
        d.hbw = []
        for hb in range(2):
            ra = max(h0, d.HBs + P * hb)
            rb = min(h1, d.HBs + P * (hb + 1))
            if ra - 1 < y0 < rb:
                dh = 0.0
            else:
                dh = min(abs(ra - y0), abs(rb - 1 - y0))
            half = wr * np.sqrt(max(0.0, s2 - (dh / hr) ** 2))
            wl = max(w0, (int(np.floor(x0 - half)) - 2) & ~1)
            wr_ = min(w1, (int(np.ceil(x0 + half)) + 4) & ~1)
            wr_ = max(wr_, wl + 2)
            d.hbw.append((wl, wr_))
        # union composite window across both h-blocks (mask is zero outside
        # each block's own [wl, wr), so fused ops over the union are exact)
        d.wlu = min(wl for wl, _ in d.hbw)
        d.wru = max(wr_ for _, wr_ in d.hbw)

        # mask over [2 blocks of 128 rows] x [wa:wb], zero outside support
        rows = (d.HBs + np.arange(2 * P, dtype=np.int64)).astype(np.float32)
        dd = (rows[:, None] - y0) ** 2 / hr ** 2 + \
             (np.arange(wa, wb, dtype=np.float32)[None, :] - x0) ** 2 / wr ** 2
        m = np.clip(np.exp(-(dd.astype(np.float32) ** np.float32(BETA)) + np.float32(1e-10)), 0.0, 1.0)
        m = np.where(dd <= np.float32(s2), m, 0.0).astype(np.float32)
        mz = np.zeros_like(m)
        for hb in range(2):
            ra = max(h0, d.HBs + P * hb) - d.HBs
            rb = min(h1, d.HBs + P * (hb + 1)) - d.HBs
            wl, wr_ = d.hbw[hb]
            mz[ra:rb, wl - wa:wr_ - wa] = m[ra:rb, wl - wa:wr_ - wa]
        m1 = np.ascontiguousarray(
            mz.reshape(2, P, d.Wt).transpose(1, 0, 2)).astype(_bf16)
        d.m_np = np.ascontiguousarray(
            np.broadcast_to(m1[:, None], (P, 2, 2, d.Wt)))

        MT = _conv_matrix(BLUR_RADII[j], KSIZES[j]).T    # MT[src, dst]
        kv = np.zeros((P, 2, d.span), np.float32)
        for k in range(2):
            r0 = d.HBs + P * k
            lo = max(0, -r0)
            hi = min(P, H - r0)
            if hi > lo:
                kv[lo:hi, k, :] = MT[r0 + lo:r0 + hi, h0:h1]
        d.kv_np = np.ascontiguousarray(kv.astype(_bf16))
        kh = np.zeros((P, 2, d.Wr), np.float32)
        for wc in range(2):
            c = d.cstarts[wc]
            kh[:, wc, :] = MT[c:c + P, w0:w1]
        # the second w-chunk overlaps the first: zero duplicated rows
        dup = d.cstarts[0] + P - d.cstarts[1]
        if dup > 0:
            kh[:dup, 1, :] = 0.0
        d.kh_np = np.ascontiguousarray(kh.astype(_bf16))
        drops.append(d)
    return drops, g0, NB, hmin, hmax


def _topo_order(drops):
    """Exact dependency DAG on (block-range x w-range) slice overlap;
    emission order = stable topological levels."""
    def _dep(i, j):
        di, dj_ = drops[i], drops[j]
        if abs(di.B0 - dj_.B0) > 1:
            return False
        ri, wi = (di.wa, di.wb), (di.w0, di.w1)
        rj, wj = (dj_.wa, dj_.wb), (dj_.w0, dj_.w1)
        for (a, b) in ((wi, rj), (ri, wj), (wi, wj)):
            if max(a[0], b[0]) < min(a[1], b[1]):
                return True
        return False

    level = [0] * NUM_DROPS
    preds = {j: [i for i in range(j) if _dep(i, j)] for j in range(NUM_DROPS)}
    for j in range(NUM_DROPS):
        for i in preds[j]:
            level[j] = max(level[j], level[i] + 1)
    # greedy order keeping every dependence >= 2 positions apart, which
    # allows a 3-deep software pipeline (comp trails om by 3 units)
    placed, remaining = [], set(range(NUM_DROPS))
    while remaining:
        cand = [j for j in sorted(remaining, key=lambda j: (level[j], j))
                if all(i not in remaining and placed.index(i) <= len(placed) - 2
                       for i in preds[j])]
        if not cand:
            cand = [j for j in sorted(remaining, key=lambda j: (level[j], j))
                    if all(i not in remaining for i in preds[j])]
        placed.append(cand[0])
        remaining.discard(cand[0])
    order = placed
    pos = {j: p for p, j in enumerate(order)}
    gap2 = all(pos[j] - pos[i] >= 2 for j in range(NUM_DROPS) for i in preds[j])
    skew = 3 if gap2 else 2
    return order, level, skew


def _store_pieces(drops, order, NB):
    """Per block: split [0,W) into up to 3 w-pieces, each tagged with the
    emission position of its LAST writer (-1 = never written)."""
    pieces = {}
    for blk in range(NB):
        last = np.full(W, -1, np.int64)
        for pos, dj in enumerate(order):
            d = drops[dj]
            if d.B0 <= blk <= d.B0 + 1:
                last[d.w0:d.w1] = pos
        runs = []
        ws = 0
        for x in range(1, W + 1):
            if x == W or last[x] != last[ws]:
                runs.append([ws, x, int(last[ws])])
                ws = x
        # merge small runs / cap count; merged run stores after max(pos)
        def _merge_once():
            k = min(range(len(runs)), key=lambda i: runs[i][1] - runs[i][0])
            if k == 0:
                k2 = 1
            elif k == len(runs) - 1:
                k2 = k - 1
            else:
                k2 = k - 1 if (runs[k - 1][1] - runs[k - 1][0]) < (runs[k + 1][1] - runs[k + 1][0]) else k + 1
            a, b = min(k, k2), max(k, k2)
            runs[a] = [runs[a][0], runs[b][1], max(runs[a][2], runs[b][2])]
            del runs[b]
        while len(runs) > 4 or min(r[1] - r[0] for r in runs) < 64:
            _merge_once()
        # even alignment
        for r in runs:
            r[0] &= ~1
        for i in range(len(runs) - 1):
            runs[i][1] = runs[i + 1][0]
        runs[-1][1] = W
        pieces[blk] = [(r[0], r[1], r[2]) for r in runs]
    return pieces


class _Balancer:
    """Greedy static load-balancer across Vector/Scalar/GpSimd with
    HW-measured per-op costs (ns): V sbuf-bf16 TT ~0.62/elem (2x mode),
    V psum-touching 1.1/elem, S copy 1.15/elem, G TT 2.0/elem.
    S (Activation) can only copy; G cannot touch PSUM."""

    def __init__(self, nc):
        self.nc = nc
        self.load = {'V': 0.0, 'S': 0.0, 'G': 0.0}

    def _pick(self, costs):
        eng, c = min(costs, key=lambda ec: self.load[ec[0]] + ec[1])
        self.load[eng] += c
        return eng

    def tt(self, op, out, a, b, fd):
        costs = [('V', fd * 0.95 + 150), ('G', fd * 2.6 + 260)]
        eng = self._pick(costs)
        e = self.nc.vector if eng == 'V' else self.nc.gpsimd
        getattr(e, 'tensor_' + op)(out, a, b)

    def copy(self, out, src, fd):
        # PSUM f32 -> SBUF (V at 1x psum rate, S activation copy)
        eng = self._pick([('V', fd * 1.15 + 200), ('S', fd * 1.1 + 200)])
        if eng == 'V':
            self.nc.vector.tensor_copy(out, src)
        else:
            self.nc.scalar.copy(out=out, in_=src)

    def bsh_mul(self, psb_sl, bshp, m_sl, t2_sl, fd, shape, dt):
        """t2 = m * psb, either via {S|V} psum-copy + {V|G} bf16 mul, or
        V direct mul from PSUM."""
        cV, cS = fd * 1.15 + 200, fd * 1.05 + 200
        mV, mG = fd * 0.65 + 150, fd * 2.6 + 260
        dV = fd * 1.15 + 200
        best, opt = None, None
        for tag, deltas in [('SV', (('S', cS), ('V', mV))),
                            ('SG', (('S', cS), ('G', mG))),
                            ('VG', (('V', cV), ('G', mG))),
                            ('D', (('V', dV),))]:
            tmp = dict(self.load)
            for e, c in deltas:
                tmp[e] += c
            key = (max(tmp.values()), sum(tmp.values()))
            if best is None or key < best:
                best, opt = key, (tag, deltas)
        tag, deltas = opt
        for e, c in deltas:
            self.load[e] += c
        if tag == 'D':
            self.nc.vector.tensor_mul(t2_sl, m_sl, psb_sl)
        else:
            bsh = bshp.tile(shape, dt, tag="Bs")
            bsh_sl = bsh[:, :, :, 0:psb_sl.shape[-1]]
            if tag[0] == 'S':
                self.nc.scalar.copy(out=bsh_sl, in_=psb_sl)
            else:
                self.nc.vector.tensor_copy(bsh_sl, psb_sl)
            e = self.nc.vector if tag[1] == 'V' else self.nc.gpsimd
            e.tensor_mul(t2_sl, m_sl, bsh_sl)


def _build_program(drops, g0, NB, hmin, hmax, order, lvl, skew, pieces):
    from contextlib import ExitStack
    from concourse import bacc, tile, mybir

    f32 = mybir.dt.float32
    bf16 = mybir.dt.bfloat16
    fp8 = mybir.dt.float8e4

    nc = bacc.Bacc("TRN2", target_bir_lowering=False, debug=False,
                   num_devices=N_CORES)

    # input params: per (pair, block-half), always 128 partitions (host
    # zero-pads rows outside [hmin, hmax)); fat DMAs = few triggers
    imgs_d = [[nc.declare_dram_parameter(f"i{g}h{hh}", [P, 2, 2, W], bf16, False)
               for hh in range(2)] for g in range(NG)]
    # output params: per (blk, piece) across ALL channels
    pu = {blk: (max(0, hmin - (g0 + 128 * blk)),
                min(P, hmax - (g0 + 128 * blk))) for blk in range(NB)}
    outs_d = {}
    for blk in range(NB):
        p0, p1 = pu[blk]
        for pi, (ws, we, _pos) in enumerate(pieces[blk]):
            outs_d[(blk, pi)] = nc.declare_dram_parameter(
                f"ob{blk}p{pi}", [p1 - p0, IC, we - ws], bf16, True)

    # drop params batched into 2 chunks by emission order (hot 2 / rest),
    # masks stored once (no jj duplication; ops broadcast via stride-0 AP)
    chunks = [order[0:2], order[2:4], order[4:]]
    kvoffs, khoffs, moffs = {}, {}, {}
    kvlen = [0] * len(chunks)
    khlen = [0] * len(chunks)
    mlen = [0] * len(chunks)
    for ci, ch in enumerate(chunks):
        for dj in ch:
            d = drops[dj]
            kvoffs[dj] = (ci, kvlen[ci])
            khoffs[dj] = (ci, khlen[ci])
            moffs[dj] = (ci, mlen[ci])
            kvlen[ci] += 2 * d.span
            khlen[ci] += 2 * d.Wr
            mlen[ci] += 4 * d.Wt
    negi_d = nc.declare_dram_parameter("negi", [P, P], bf16, False)
    pchunks = []
    for ci, ch in enumerate(chunks):
        pchunks.append((
            nc.declare_dram_parameter(f"mc{ci}", [P, mlen[ci]], bf16, False),
            nc.declare_dram_parameter(f"kvc{ci}", [P, kvlen[ci]], bf16, False),
            nc.declare_dram_parameter(f"khc{ci}", [P, khlen[ci]], bf16, False)))

    bal = _Balancer(nc)

    with tile.TileContext(nc) as tc, ExitStack() as ctx:
        outp = ctx.enter_context(tc.tile_pool(name="out_state", bufs=1))
        out_s = outp.tile([P, IC, NB, W], bf16, name="state", tag="state")
        dp = ctx.enter_context(tc.tile_pool(name="dropin", bufs=1))
        omp = ctx.enter_context(tc.tile_pool(name="omq", bufs=8))
        vtp = ctx.enter_context(tc.tile_pool(name="vts", bufs=8))
        bshp = ctx.enter_context(tc.tile_pool(name="bsh", bufs=8))
        ppa = ctx.enter_context(tc.tile_pool(name="psa", bufs=2, space="PSUM"))
        ppb = ctx.enter_context(tc.tile_pool(name="psb", bufs=2, space="PSUM"))

        # ---- PE warm-up: matmuls on a zeroed tile span the load window
        wt = dp.tile([P, 512], bf16, tag="warm")
        nc.gpsimd.memset(wt[:], 0)
        warm = ppa.tile([P, 2, 2, 256], f32, tag="psa")
        for i in range(26):
            nc.tensor.matmul(warm[:, 0, 0, 0:256], lhsT=wt[:, 0:P],
                             rhs=wt[:, 0:256], start=True, stop=True)
        # pre-zero the vt ring so pass-B stationaries never read NaN garbage
        for i in range(4):
            v0 = vtp.tile([P, 2, 2, 256], bf16, tag="vt", bufs=4)
            (nc.vector if i % 2 else nc.gpsimd).memset(v0[:], 0)

        # ---- loads: params chunk0 on scalar; imgs pair-major on sync so
        # each drop chain starts as its blocks arrive; later chunks follow
        ptiles = []
        for ci, ch in enumerate(chunks):
            ptiles.append((
                dp.tile([P, mlen[ci]], bf16, tag=f"mc{ci}", name=f"mc{ci}"),
                dp.tile([P, kvlen[ci]], bf16, tag=f"kvc{ci}", name=f"kvc{ci}"),
                dp.tile([P, khlen[ci]], bf16, tag=f"khc{ci}", name=f"khc{ci}")))
        # single (sync) queue so bytes arrive strictly in first-use order:
        # hot params (first 4 drops), img halves, then the remaining params
        negi = dp.tile([P, P], bf16, tag="negi", name="negi")
        nc.sync.dma_start(out=negi[:], in_=negi_d.ap()[:])
        for t, pd in zip(ptiles[0], pchunks[0]):
            nc.sync.dma_start(out=t[:], in_=pd.ap()[:])
        for hh in range(2):
            for g in range(NG):
                nc.sync.dma_start(
                    out=out_s[:, 2 * g:2 * g + 2, 2 * hh:2 * hh + 2, :],
                    in_=imgs_d[g][hh].ap()[:])
        for ci in (1, 2):
            for t, pd in zip(ptiles[ci], pchunks[ci]):
                nc.sync.dma_start(out=t[:], in_=pd.ap()[:])

        # position of each drop in emission order, for store scheduling
        pos_of = {dj: pos for pos, dj in enumerate(order)}
        store_after = {}
        for blk in range(NB):
            for pi, (ws, we, pos) in enumerate(pieces[blk]):
                store_after.setdefault(max(pos, 0), []).append((blk, pi, ws, we))

        # ---- drops: software-pipelined at (drop, pair) granularity so no
        # engine FIFO blocks at its head and PSUM rings (2 bufs each) are
        # recycled only after their reader is emitted.
        #   iteration t: comp(u[t-2]) -> evict+q+passB(u[t-1]) -> om+passA(u[t])
        waves = {}
        for dj in order:
            waves.setdefault(lvl[dj], []).append(dj)

        class _U:
            pass

        def stage1(dj, g):
            u = _U()
            d = drops[dj]
            ci = next(ii for ii, ch in enumerate(chunks) if dj in ch)
            u.d, u.g, u.dj = d, g, dj
            u.i = chunks[ci].index(dj)
            u.mt, u.kvt, u.kht = ptiles[ci]
            _, u.kvo = kvoffs[dj]
            _, u.kho = khoffs[dj]
            _, mo = moffs[dj]
            # mask [P, 2(jj), 2(hb), Wt] materialized (plain strided slices
            # keep the DVE 2x fast path that broadcast APs lose)
            u.m2 = u.mt[:, mo:mo + 4 * d.Wt].rearrange(
                "p (j k w) -> p j k w", j=2, k=2)
            sl = out_s[:, 2 * g:2 * g + 2, d.B0:d.B0 + 2, d.wa:d.wb]
            u.om = omp.tile([P, 2, 2, 256], bf16, tag="om", bufs=4)
            bal.tt('mul', u.om[:, :, :, 0:d.Wt], u.m2, sl, 4 * d.Wt)
            u.psa = ppa.tile([P, 2, 2, 256], f32, tag="psa", bufs=2)
            for wc in range(2):
                coff = d.cstarts[wc] - d.wa
                for jj in range(2):
                    for k in range(2):
                        a, b = d.bandsA[k]
                        nc.tensor.matmul(
                            u.psa[:, jj, wc, a:b],
                            lhsT=u.om[:, jj, k, coff:coff + P],
                            rhs=u.kvt[:, u.kvo + k * d.span + a:u.kvo + k * d.span + b],
                            start=(k == 0), stop=(k == 1))
            return u

        def stage2(u):
            d, g = u.d, u.g
            au, bu = d.wlu - d.w0, d.wru - d.w0
            u.vt = vtp.tile([P, 2, 2, 256], bf16, tag="vt", bufs=4)
            bal.copy(u.vt[:, :, :, d.voff:d.voff + d.span],
                     u.psa[:, :, :, 0:d.span], 4 * d.span)
            u.psb = ppb.tile([P, 2, 2, 256], f32, tag="psb", bufs=2)
            # wc0 streams the full union band (start=True zeroes psb there);
            # wc1 accumulates only its true kh support [c1-p, bu)
            a1 = max(au, d.cstarts[1] - d.p - d.w0)
            for jj in range(2):
                for hb in range(2):
                    nc.tensor.matmul(
                        u.psb[:, jj, hb, au:bu],
                        lhsT=u.vt[:, jj, 0, hb * P:(hb + 1) * P],
                        rhs=u.kht[:, u.kho + au:u.kho + bu],
                        start=True, stop=False)
                    nc.tensor.matmul(
                        u.psb[:, jj, hb, a1:bu],
                        lhsT=u.vt[:, jj, 1, hb * P:(hb + 1) * P],
                        rhs=u.kht[:, u.kho + d.Wr + a1:u.kho + d.Wr + bu],
                        start=False, stop=False)
                    # psb = B - out: -I @ state makes the composite a pure
                    # masked accumulate (q precompute eliminated)
                    nc.tensor.matmul(
                        u.psb[:, jj, hb, au:bu],
                        lhsT=negi[:],
                        rhs=out_s[:, 2 * g + jj, d.B0 + hb, d.wlu:d.wru],
                        start=False, stop=True)

        def stage3(u):
            d, g = u.d, u.g
            au, bu = d.wlu - d.w0, d.wru - d.w0
            widu = bu - au
            t2 = bshp.tile([P, 2, 2, 256], bf16, tag="t2", bufs=4)
            bal.bsh_mul(u.psb[:, :, :, au:bu], bshp,
                        u.m2[:, :, :, d.wlu - d.wa:d.wru - d.wa],
                        t2[:, :, :, 0:widu], 4 * widu, [P, 2, 2, 256], bf16)
            osl = out_s[:, 2 * g:2 * g + 2, d.B0:d.B0 + 2, d.wlu:d.wru]
            bal.tt('add', osl, osl, t2[:, :, :, 0:widu], 4 * widu)
            # store each channel-pair's slice as soon as ITS composite of
            # the piece's last-writer drop lands (shrinks the final drain)
            for (blk, pi, ws, we) in store_after.get(pos_of[u.dj], []):
                p0, p1 = pu[blk]
                nc.sync.dma_start(
                    out=outs_d[(blk, pi)].ap()[:, 2 * g:2 * g + 2],
                    in_=out_s[p0:p1, 2 * g:2 * g + 2, blk, ws:we])

        units = [(dj, g) for dj in order for g in range(NG)]
        ring = []
        for t in range(len(units) + skew):
            if t >= skew and t - skew < len(units):
                stage3(ring[t - skew])
            if t >= 1 and t - 1 < len(units):
                stage2(ring[t - 1])
            if t < len(units):
                ring.append(stage1(*units[t]))
    nc.compile()
    print("balancer loads (us):",
          {k: round(v / 1000, 1) for k, v in bal.load.items()})
    return nc


_CACHE = {}


def _get_program(positions, radius):
    key = (np.asarray(positions, np.float32).tobytes(),
           np.asarray(radius, np.float32).tobytes())
    if key not in _CACHE:
        drops, g0, NB, hmin, hmax = _drop_meta(positions, radius)
        order, level, skew = _topo_order(drops)
        print("emission order:", order, "levels:", level, "skew:", skew)
        pieces = _store_pieces(drops, order, NB)
        nc = _build_program(drops, g0, NB, hmin, hmax, order, level, skew, pieces)
        _CACHE[key] = (nc, drops, g0, NB, hmin, hmax, order, pieces)
    return _CACHE[key]


def kernel(img, positions, radius, _want_trace=False, **_kw):
    from concourse.bass_utils import run_bass_kernel_spmd
    img = np.asarray(img, np.float32)
    assert img.shape == (B_TOTAL, C, H, W)
    nc, drops, g0, NB, hmin, hmax, order, pieces = _get_program(positions, radius)

    # pack rows [g0, g0+NB*128) to [p, pair(2), w] per (core, pair, blk), bf16,
    # zero-padded outside [hmin, hmax)
    rows_lo, rows_hi = hmin, hmax
    imgb = np.zeros((N_CORES, IC, NB * P, W), _bf16)
    src = img.reshape(N_CORES, IC, H, W)
    imgb[:, :, rows_lo - g0:rows_hi - g0, :] = src[:, :, rows_lo:rows_hi, :].astype(_bf16)
    packed = np.ascontiguousarray(
        imgb.reshape(N_CORES, IC, NB, P, W).transpose(0, 3, 1, 2, 4))

    chunks = [order[0:2], order[2:4], order[4:]]
    base = {}
    for ci, ch in enumerate(chunks):
        base[f"mc{ci}"] = np.ascontiguousarray(np.concatenate(
            [drops[dj].m_np.reshape(P, -1) for dj in ch], axis=1))
        base[f"kvc{ci}"] = np.ascontiguousarray(np.concatenate(
            [drops[dj].kv_np.reshape(P, -1) for dj in ch], axis=1))
        base[f"khc{ci}"] = np.ascontiguousarray(np.concatenate(
            [drops[dj].kh_np.reshape(P, -1) for dj in ch], axis=1))
    base["negi"] = np.ascontiguousarray((-np.eye(P)).astype(_bf16))
    in_maps = []
    for i in range(N_CORES):
        mp = dict(base)
        for g in range(NG):
            for hh in range(2):
                mp[f"i{g}h{hh}"] = np.ascontiguousarray(
                    packed[i][:, 2 * g:2 * g + 2, 2 * hh:2 * hh + 2, :])
        in_maps.append(mp)
    res = run_bass_kernel_spmd(nc, in_maps, core_ids=list(range(N_CORES)),
                               trace=_want_trace)
    out = img.copy()
    pu = {blk: (max(0, hmin - (g0 + 128 * blk)),
                min(P, hmax - (g0 + 128 * blk))) for blk in range(NB)}
    for i in range(N_CORES):
        oc = out.reshape(N_CORES, IC, H, W)
        for blk in range(NB):
            p0, p1 = pu[blk]
            r0 = g0 + 128 * blk + p0
            for pi, (ws, we, _pos) in enumerate(pieces[blk]):
                blkres = res.results[i][f"ob{blk}p{pi}"]
                # [Pu, IC, wlen] -> rows r0..r0+Pu
                oc[i, :, r0:r0 + (p1 - p0), ws:we] = \
                    blkres.transpose(1, 0, 2).astype(np.float32)
    if _want_trace:
        return out, res
    return out
